# revision 11
# baseline (speedup 1.0000x reference)
"""Causal single-head attention on 8 trn2 NeuronCores.

B=4, S=2048, D_MODEL=1024, D_HEAD=64, fp32 in/out.

Sharding: 2 cores per batch. Core half h=0 owns query tiles {0..3,12..15}
(rows 0:512, 1536:2048), h=1 owns {4..11} (rows 512:1536); both own 68
causal 128x128 blocks. The host feeds each core its batch's embeddings
already TRANSPOSED to E^T [dm, s] in bf16 with columns permuted so own
query rows come first - no on-device transposes/casts of E at all.

Per-core pipeline (identical SPMD program, all matmuls bf16):
  Warmup N=512 matmuls on a scratch tile open the PE HAM clock gate
  (1.2->2.4 GHz) while the first input DMA is in flight. Weights land in
  their own first dma_start so projections start ~2us earlier; the tri
  mask is a single shared 128x128 diagonal block (48KB with the identity,
  vs 540KB of per-tile tails - off-diagonal tail cols need no mask).
  Projections per 512-col chunk of E^T: one [Wv|Wk]-packed pass (V^T on
  PSUM rows 0:64, K^T on rows 64:128) plus, for the core's own 2 chunks,
  a Wq/8 pass targeting PSUM rows 64:128. Q^T and K^T both live on SBUF
  partitions 64:128 so score matmuls satisfy the shared-base-partition
  rule; V tiles are PE-transposed into Vp [128k, 16, 65] with a ones
  column (softmax denominator). With zero biases ALL PSUM->SBUF
  projection copies are bias-free (ACT engine early, DVE later), so no
  score matmul ever waits on the bias DMA.
  Attention over local key tiles kt, with score/exp/mask/PV regions
  trimmed to the causal need:
    kt 0..3  : cols [kt*128:1024] (slot0 tri tail + slot1 full), one exp
    kt 4..7  : slot1 tri tail only
    kt 8..11 : both slots; slot0 killed by a 0/-30000 exp bias on h=0
    kt 12..15: slot1 only; per-core 0/-30000 exp bias kills it on h=1
  PV accumulates out^T [65, 512] per slot in PSUM (col 64 = sum exp) and
  is DMA'd PSUM->HBM directly; the host does the final divide +
  transpose + scatter.
"""

import sys

if "/opt/trn_rl_repo" not in sys.path:
    sys.path.insert(0, "/opt/trn_rl_repo")

import numpy as np

B, S, D, H = 4, 2048, 1024, 64
P = 128
KO = D // P          # 8 dmodel chunks
NT = S // P          # 16 seq tiles
NEG = -30000.0


def _halves():
    return [[(0, 512), (1536, 2048)], [(512, 1536)]]


def _build_program(zb):
    import concourse.bacc as bacc
    import concourse.mybir as mybir
    import concourse.tile as tile

    f32 = mybir.dt.float32
    bf16 = mybir.dt.bfloat16
    AF = mybir.ActivationFunctionType
    ALU = mybir.AluOpType

    nc = bacc.Bacc()
    # et layout [chunk, partition, KO*512]: 8 KB contiguous per partition
    # per chunk -> big DMA descriptors (1 KB descriptors run ~21 GB/s/queue)
    et = nc.declare_dram_parameter("et", [4, P, KO * 512], bf16, isOutput=False)
    # weights + ET chunk 0: per partition cols 0:1536 = [Wv|Wk|Wq/8] x 8 ko
    # (192 each), cols 1536:5632 = chunk0
    wc0 = nc.declare_dram_parameter("wc0", [P, 1536 + 4096], bf16, isOutput=False)
    # cols: bq/8 | bk | g8 | g12n | bv (bv only rows 0:64 meaningful)
    bias4 = nc.declare_dram_parameter("bias4", [P, 5], f32, isOutput=False)
    # cols 0:128 = shared tri diag mask, cols 128:192 = identity (rows 0:64)
    mi = nc.declare_dram_parameter("mi", [P, P + H], bf16, isOutput=False)
    out = nc.declare_dram_parameter("out", [H + 1, 1024], f32, isOutput=True)

    from contextlib import ExitStack

    with tile.TileContext(nc) as tc, ExitStack() as ctx:
        cpool = ctx.enter_context(tc.tile_pool(name="const", bufs=1))
        vtp = ctx.enter_context(tc.tile_pool(name="vt", bufs=2))
        ptp = ctx.enter_context(tc.tile_pool(name="pt", bufs=10))
        psb = ctx.enter_context(tc.tile_pool(name="psb", bufs=2, space="PSUM"))

        # --- input DMAs, split across BOTH hardware DGE rings (Sync + Act)
        # so weights/chunk0 stream in parallel with ET 1..3. Within a ring
        # transfers land in issue order; critical pieces go first.
        wc_sb = cpool.tile([P, 1536 + 4096], bf16, tag="wc0")
        nc.scalar.dma_start(wc_sb[:, 0:1536], wc0[:, 0:1536])
        nc.scalar.dma_start(wc_sb[:, 1536:4096], wc0[:, 1536:4096])
        nc.scalar.dma_start(wc_sb[:, 4096:5632], wc0[:, 4096:5632])
        mi_sb = cpool.tile([P, P + H], bf16, tag="mi")
        nc.sync.dma_start(mi_sb[:], mi[:])
        bias_sb = cpool.tile([P, 5], f32, tag="bias4")
        nc.sync.dma_start(bias_sb[:], bias4[:])
        # [partition, chunk, ko, 512]; chunk 0 lives in wc_sb instead
        ET = cpool.tile([P, 4, KO, 512], bf16, tag="ET")
        nc.sync.dma_start(ET[:, 1, :, :], et[1, :, :])
        nc.scalar.dma_start(ET[:, 2, :, :], et[2, :, :])
        nc.sync.dma_start(ET[:, 3, :, :], et[3, :, :])

        def w_ap(ko, a, b):      # weight cols a:b of ko-th 192-block
            return wc_sb[:, ko * 192 + a:ko * 192 + b]

        def et_ap(cc, ko):       # ET chunk cc, ko-th 512-col block
            if cc == 0:
                return wc_sb[:, 1536 + ko * 512:1536 + (ko + 1) * 512]
            return ET[:, cc, ko, :]

        bq_sb = bias_sb[:, 0:1]
        bk_sb = bias_sb[:, 1:2]
        g8_sb = bias_sb[:, 2:3]
        g12_sb = bias_sb[:, 3:4]
        bv_sb = bias_sb[:H, 4:5]
        tri_sb = mi_sb[:, 0:P]
        id_sb = mi_sb[:H, P:P + H]

        # Q^T and K^T both live on partitions 64:128 (matmul requires lhsT
        # and rhs to share a base partition; the packed [Wv|Wk] projection
        # puts K^T on PSUM rows 64:128 and DVE copies cannot shift rows).
        QT = cpool.tile([P, 1024], bf16, tag="QT")
        KT = cpool.tile([P, S], bf16, tag="KT")
        Vp = cpool.tile([P, NT, H + 1], bf16, tag="Vp")
        o_sb = cpool.tile([H + 1, 1024], f32, tag="osb")
        # HAM warmup scratch: memset FIRST on DVE so dependency-free N=512
        # matmuls start as early as possible and open the clock gate
        # (1.2 -> 2.4 GHz) before the weights DMA lands.
        wtile = cpool.tile([P, 512], bf16, tag="warm")
        nc.vector.memset(wtile[:], 0.0)
        nc.vector.memset(Vp[:, :, H:H + 1], 1.0)

        def vtranspose(vt, cc):
            for t in range(4):
                kt = cc * 4 + t
                pvt = psb.tile([P, H], bf16, tag="pj", name=f"pvt_{kt}")
                nc.tensor.transpose(
                    pvt[:], vt[:, t * P:(t + 1) * P], id_sb[:]
                )
                nc.vector.tensor_copy(Vp[:, kt, :H], pvt[:])

        vts = [None] * 4

        def pcopy(dst, src_ap, bias, eng):
            # PSUM->SBUF projection copy; with zero biases no copy reads
            # the bias DMA (early ones on the otherwise-idle ACT engine,
            # later ones on DVE), so scores never stall on it
            if zb:
                if eng == "act":
                    nc.scalar.activation(dst, src_ap, AF.Copy)
                else:
                    nc.vector.tensor_copy(dst, src_ap)
            else:
                nc.vector.tensor_scalar_add(dst, src_ap, bias)

        def vk_chunk(cc):
            # one pass of the ET chunk computes V^T (rows 0:64) + K^T (64:128)
            ps = psb.tile([P, 512], f32, tag="pj", name=f"vk_ps_{cc}")
            for ko in range(KO):
                nc.tensor.matmul(
                    ps[:], w_ap(ko, 0, 128), et_ap(cc, ko),
                    start=(ko == 0), stop=(ko == KO - 1),
                )
            eng = "act" if cc < 1 else "dve"
            pcopy(
                KT[H:P, cc * 512:(cc + 1) * 512], ps[H:P, :], bk_sb[H:P], eng
            )
            vt = vtp.tile([H, 512], bf16, tag="vt", name=f"vt_{cc}")
            pcopy(vt[:], ps[:H, :], bv_sb[:], eng)
            vts[cc] = vt

        def q_chunk(cc):
            # M=64 matmul targeting PSUM rows 64:128 so Q^T lands at base 64
            ps = psb.tile([P, 512], f32, tag="pj", name=f"q_ps_{cc}")
            for ko in range(KO):
                nc.tensor.matmul(
                    ps[H:P, :], w_ap(ko, 128, 192), et_ap(cc, ko),
                    start=(ko == 0), stop=(ko == KO - 1),
                )
            pcopy(
                QT[H:P, cc * 512:(cc + 1) * 512], ps[H:P, :], bq_sb[H:P],
                "act" if cc == 0 else "dve",
            )

        # --- attention ---
        outT0 = psb.tile([P, 512], f32, tag="os0", bufs=1)
        outT1 = psb.tile([P, 512], f32, tag="os1", bufs=1)

        # pvs[kt] = list of (outT, col0, rhs_ap) PV pieces for that key tile
        pvs = [None] * NT

        def tri_mult(pt, c0):
            # only the 128-col diagonal block needs masking; the rest of a
            # causal tail is all-ones
            nc.vector.tensor_tensor(
                pt[:, c0:c0 + P], pt[:, c0:c0 + P], tri_sb, ALU.mult
            )

        def sc(kt):
            # score regions trimmed to the causal need:
            #  kt 0..3  : cols [kt*128 : 1024] (slot0 tri tail + slot1 full)
            #  kt 4..7  : slot1 tri tail, cols [(kt-4)*128 : 512] of slot1
            #  kt 8..11 : both slots full; slot0 multiplied by 0/1 gate
            #  kt 12..15: slot1 full, exp-bias gated
            ps = psb.tile(
                [P, 1024], f32, tag="sc", name=f"sc_{kt}", bufs=2
            )
            kblk = KT[H:P, kt * P:(kt + 1) * P]
            pt = ptp.tile([P, 1024], bf16, tag="pt", name=f"pt_{kt}")
            if kt < 4 or (8 <= kt < 12):
                c0 = kt * P if kt < 4 else 0
                nc.tensor.matmul(
                    ps[:, c0:512], kblk, QT[H:P, c0:512],
                    start=True, stop=True, skip_group_check=True,
                )
                nc.tensor.matmul(
                    ps[:, 512:1024], kblk, QT[H:P, 512:1024],
                    start=True, stop=True, skip_group_check=True,
                )
                # two half-exps: slot0's PV can start while slot1 still exps;
                # for kt 8..11 the per-core 0/-30000 exp bias zeroes slot0 on
                # the core whose slot0 queries precede these keys
                if kt < 4:
                    nc.scalar.activation(pt[:, c0:512], ps[:, c0:512], AF.Exp)
                    tri_mult(pt, c0)
                else:
                    nc.scalar.activation(
                        pt[:, 0:512], ps[:, 0:512], AF.Exp, bias=g8_sb[:]
                    )
                nc.scalar.activation(
                    pt[:, 512:1024], ps[:, 512:1024], AF.Exp
                )
                pvs[kt] = [
                    (outT0, c0, pt[:, c0:512]),
                    (outT1, 0, pt[:, 512:1024]),
                ]
            else:
                c0 = (kt - 4) * P if kt < 12 else 0
                n = 512 - c0
                nc.tensor.matmul(
                    ps[:, 0:n], kblk, QT[H:P, 512 + c0:1024],
                    start=True, stop=True, skip_group_check=True,
                )
                if kt >= 12:
                    nc.scalar.activation(
                        pt[:, 0:n], ps[:, 0:n], AF.Exp, bias=g12_sb[:]
                    )
                else:
                    nc.scalar.activation(pt[:, 0:n], ps[:, 0:n], AF.Exp)
                    tri_mult(pt, 0)
                pvs[kt] = [(outT1, c0, pt[:, 0:n])]

        def pv(kt, stop0=False, stop1=False):
            for outT, c0, rhs in pvs[kt]:
                nc.tensor.matmul(
                    outT[:H + 1, c0:512], Vp[:, kt, :], rhs,
                    start=(kt == 0),
                    stop=(stop0 if outT is outT0 else stop1),
                    skip_group_check=True,
                )

        # --- emission order = per-engine FIFO order; hand-pipelined so PE
        # never waits on ACT/DVE and ACT starts exping early ---
        # HAM warmup: dependency-free N=512 matmuls on a zeroed scratch
        # tile run back-to-back from ~7.3us, opening the clock gate before
        # the first projection matmul (~11us). Results go to dead psum.
        for i in range(13):
            wps = psb.tile([P, 512], f32, tag="pj", name=f"warm_{i}")
            nc.tensor.matmul(
                wps[:], wtile[:, 0:P], wtile[:],
                start=True, stop=True, skip_group_check=True,
            )

        # kt 0 and 1 split in half-scores: the slot0 halves (which need
        # only Q chunk 0) issue before q_chunk(1), so ACT starts exping
        # ~2us earlier in the proj->attention transition
        eps = {}
        ept = {}

        def sc_half_a(kt):
            c0 = kt * P
            ps = psb.tile([P, 1024], f32, tag="sc", name=f"sc_{kt}", bufs=2)
            pt = ptp.tile([P, 1024], bf16, tag="pt", name=f"pt_{kt}")
            eps[kt], ept[kt] = ps, pt
            nc.tensor.matmul(
                ps[:, c0:512], KT[H:P, kt * P:(kt + 1) * P], QT[H:P, c0:512],
                start=True, stop=True, skip_group_check=True,
            )
            nc.scalar.activation(pt[:, c0:512], ps[:, c0:512], AF.Exp)
            tri_mult(pt, c0)

        def sc_half_b(kt):
            ps, pt = eps[kt], ept[kt]
            nc.tensor.matmul(
                ps[:, 512:1024], KT[H:P, kt * P:(kt + 1) * P],
                QT[H:P, 512:1024],
                start=True, stop=True, skip_group_check=True,
            )
            nc.scalar.activation(pt[:, 512:1024], ps[:, 512:1024], AF.Exp)
            pvs[kt] = [
                (outT0, kt * P, pt[:, kt * P:512]),
                (outT1, 0, pt[:, 512:1024]),
            ]

        def sc67():
            # kt 6 (256 cols) and 7 (128 cols) share one psum bank + exp
            ps = psb.tile([P, 1024], f32, tag="sc", name="sc_67", bufs=2)
            pt = ptp.tile([P, 1024], bf16, tag="pt", name="pt_67")
            nc.tensor.matmul(
                ps[:, 0:256], KT[H:P, 6 * P:7 * P], QT[H:P, 768:1024],
                start=True, stop=True, skip_group_check=True,
            )
            nc.tensor.matmul(
                ps[:, 256:384], KT[H:P, 7 * P:8 * P], QT[H:P, 896:1024],
                start=True, stop=True, skip_group_check=True,
            )
            nc.scalar.activation(pt[:, 0:384], ps[:, 0:384], AF.Exp)
            tri_mult(pt, 0)
            tri_mult(pt, 256)
            pvs[6] = [(outT1, 256, pt[:, 0:256])]
            pvs[7] = [(outT1, 384, pt[:, 256:384])]

        vk_chunk(0)
        q_chunk(0)
        sc_half_a(0)
        sc_half_a(1)
        q_chunk(1)
        sc_half_b(0)
        sc_half_b(1)
        sc(2)
        sc(3)
        vk_chunk(1)
        sc(4)
        sc(5)
        sc67()
        vk_chunk(2)
        sc(8)
        sc(9)
        vtranspose(vts[0], 0)
        vtranspose(vts[1], 1)
        pv(0)
        pv(1)
        pv(2)
        pv(3)
        pv(4)
        pv(5)
        vk_chunk(3)
        vtranspose(vts[2], 2)
        sc(10)
        pv(6)
        sc(11)
        pv(8)
        sc(12)
        pv(9)
        sc(13)
        vtranspose(vts[3], 3)
        sc(14)
        sc(15)
        pv(10)
        pv(11, stop0=True)
        nc.vector.tensor_copy(o_sb[:, 0:512], outT0[:H + 1, :])
        nc.sync.dma_start(out[:, 0:512], o_sb[:, 0:512])
        pv(12)
        pv(13)
        pv(14)
        pv(15)
        pv(7, stop1=True)
        nc.vector.tensor_copy(o_sb[:, 512:1024], outT1[:H + 1, :])
        nc.sync.dma_start(out[:, 512:1024], o_sb[:, 512:1024])

    nc.finalize()
    return nc


_CACHED = None


def _get_program(zb):
    global _CACHED
    if _CACHED is None or _CACHED[0] != zb:
        _CACHED = (zb, _build_program(zb))
    return _CACHED[1]


def _host_inputs(embeddings, Wq, bq, Wk, bk, Wv, bv):
    import ml_dtypes

    bf16 = ml_dtypes.bfloat16
    halves = _halves()
    # shared multiplicative tri diag mask: 1 where c >= k; plus identity
    tri = np.zeros((P, P), np.float32)
    for k in range(P):
        tri[k, k:] = 1.0
    ident = np.zeros((P, H), np.float32)
    ident[:H] = np.eye(H, dtype=np.float32)
    mi = np.ascontiguousarray(
        np.concatenate([tri, ident], axis=1)
    ).astype(bf16)

    def wlay(w):
        return np.asarray(w, np.float32).reshape(KO, P, H).transpose(1, 0, 2)

    wq8l = wlay(Wq) / 8.0
    wkl = wlay(Wk)
    wvl = wlay(Wv)
    wts = np.concatenate([wvl, wkl, wq8l], axis=2).reshape(P, 1536)
    bqf = np.asarray(bq, np.float32) / 8.0
    bkf = np.asarray(bk, np.float32)
    bvf = np.asarray(bv, np.float32)
    z64 = np.zeros(H, np.float32)
    bq8P = np.concatenate([z64, bqf])
    bkP = np.concatenate([z64, bkf])
    bvP = np.concatenate([bvf, z64])

    in_maps = []
    perms = []
    for c in range(8):
        b, h = c // 2, c % 2
        own = halves[h]
        other = halves[1 - h]
        rows = np.concatenate(
            [np.arange(a, z) for a, z in own] + [np.arange(a, z) for a, z in other]
        )
        perms.append(rows)
        ep = embeddings[b][rows]                      # [S, D] f32, permuted
        etl = np.ascontiguousarray(
            ep.T.reshape(KO, P, 4, 512).transpose(2, 1, 0, 3)
        ).astype(bf16).reshape(4, P, KO * 512)        # [cc, p, ko*512]
        g8v = np.full(P, 0.0 if h == 1 else NEG, np.float32)
        g12v = np.full(P, NEG if h == 1 else 0.0, np.float32)
        bias4 = np.ascontiguousarray(
            np.stack([bq8P, bkP, g8v, g12v, bvP], axis=1)
        )
        wc0l = np.ascontiguousarray(
            np.concatenate([wts, etl[0]], axis=1)
        ).astype(bf16)
        in_maps.append({
            "et": etl, "wc0": wc0l, "bias4": bias4, "mi": mi,
        })
    return in_maps, perms


def _run(embeddings, Wq, bq, Wk, bk, Wv, bv, trace=False):
    from concourse.bass_utils import run_bass_kernel_spmd

    zb = (
        not np.any(np.asarray(bq)) and not np.any(np.asarray(bk))
        and not np.any(np.asarray(bv))
    )
    nc = _get_program(zb)
    in_maps, perms = _host_inputs(embeddings, Wq, bq, Wk, bk, Wv, bv)
    res = run_bass_kernel_spmd(
        nc, in_maps, core_ids=list(range(8)), trace=trace,
        trace_cores=list(range(8)) if trace else None,
    )
    full = np.empty((B, S, H), np.float32)
    for c in range(8):
        b = c // 2
        o = res.results[c]["out"]                     # [65, 1024] f32
        full[b, perms[c][:1024]] = (o[:H] / o[H:H + 1]).T
    return full, res


def kernel(embeddings, Wq, bq, Wk, bk, Wv, bv):
    full, _ = _run(
        np.asarray(embeddings, np.float32), Wq, bq, Wk, bk, Wv, bv, trace=False
    )
    return full


# revision 13
# speedup vs baseline: 1.0193x; 1.0193x over previous
"""Causal single-head attention on 8 trn2 NeuronCores.

B=4, S=2048, D_MODEL=1024, D_HEAD=64, fp32 in/out.

Sharding: 2 cores per batch. Core half h=0 owns query tiles {0..3,12..15}
(rows 0:512, 1536:2048), h=1 owns {4..11} (rows 512:1536); both own 68
causal 128x128 blocks. The host feeds each core its batch's embeddings
already TRANSPOSED to E^T [dm, s] in bf16 with columns permuted so own
query rows come first - no on-device transposes/casts of E at all.

Per-core pipeline (identical SPMD program, all matmuls bf16):
  Warmup N=512 matmuls on a scratch tile open the PE HAM clock gate
  (1.2->2.4 GHz) while the first input DMA is in flight. Weights land in
  their own first dma_start so projections start ~2us earlier; the tri
  mask is a single shared 128x128 diagonal block (48KB with the identity,
  vs 540KB of per-tile tails - off-diagonal tail cols need no mask).
  Projections per 512-col chunk of E^T: one [Wv|Wk]-packed pass (V^T on
  PSUM rows 0:64, K^T on rows 64:128) plus, for the core's own 2 chunks,
  a Wq/8 pass targeting PSUM rows 64:128. Q^T and K^T both live on SBUF
  partitions 64:128 so score matmuls satisfy the shared-base-partition
  rule; V tiles are PE-transposed into Vp [128k, 16, 65] with a ones
  column (softmax denominator). With zero biases ALL PSUM->SBUF
  projection copies are bias-free (ACT engine early, DVE later), so no
  score matmul ever waits on the bias DMA.
  Attention over local key tiles kt, with score/exp/mask/PV regions
  trimmed to the causal need:
    kt 0..3  : cols [kt*128:1024] (slot0 tri tail + slot1 full), one exp
    kt 4..7  : slot1 tri tail only
    kt 8..11 : both slots; slot0 killed by a 0/-30000 exp bias on h=0
    kt 12..15: slot1 only; per-core 0/-30000 exp bias kills it on h=1
  PV accumulates out^T [65, 512] per slot in PSUM (col 64 = sum exp) and
  is DMA'd PSUM->HBM directly; the host does the final divide +
  transpose + scatter.
"""

import sys

if "/opt/trn_rl_repo" not in sys.path:
    sys.path.insert(0, "/opt/trn_rl_repo")

import numpy as np

B, S, D, H = 4, 2048, 1024, 64
P = 128
KO = D // P          # 8 dmodel chunks
NT = S // P          # 16 seq tiles
NEG = -30000.0


def _halves():
    return [[(0, 512), (1536, 2048)], [(512, 1536)]]


def _build_program(zb):
    import concourse.bacc as bacc
    import concourse.mybir as mybir
    import concourse.tile as tile

    f32 = mybir.dt.float32
    bf16 = mybir.dt.bfloat16
    AF = mybir.ActivationFunctionType
    ALU = mybir.AluOpType

    nc = bacc.Bacc()
    # et layout [chunk, partition, KO*512]: 8 KB contiguous per partition
    # per chunk -> big DMA descriptors (1 KB descriptors run ~21 GB/s/queue)
    et = nc.declare_dram_parameter("et", [4, P, KO * 512], bf16, isOutput=False)
    # weights + ET chunk 0: per partition cols 0:1536 = [Wv|Wk|Wq/8] x 8 ko
    # (192 each), cols 1536:5632 = chunk0
    wc0 = nc.declare_dram_parameter("wc0", [P, 1536 + 4096], bf16, isOutput=False)
    # cols: bq/8 | bk | g8 | g12n | bv (bv only rows 0:64 meaningful)
    bias4 = nc.declare_dram_parameter("bias4", [P, 5], f32, isOutput=False)
    # cols 0:128 = shared tri diag mask, cols 128:192 = identity (rows 0:64)
    mi = nc.declare_dram_parameter("mi", [P, P + H], bf16, isOutput=False)
    out = nc.declare_dram_parameter("out", [H + 1, 1024], f32, isOutput=True)

    from contextlib import ExitStack

    with tile.TileContext(nc) as tc, ExitStack() as ctx:
        cpool = ctx.enter_context(tc.tile_pool(name="const", bufs=1))
        vtp = ctx.enter_context(tc.tile_pool(name="vt", bufs=2))
        ptp = ctx.enter_context(tc.tile_pool(name="pt", bufs=10))
        psb = ctx.enter_context(tc.tile_pool(name="psb", bufs=2, space="PSUM"))

        # --- input DMAs, split across BOTH hardware DGE rings (Sync + Act)
        # so weights/chunk0 stream in parallel with ET 1..3. Within a ring
        # transfers land in issue order; critical pieces go first.
        wc_sb = cpool.tile([P, 1536 + 4096], bf16, tag="wc0")
        nc.sync.dma_start(wc_sb[:, 0:1536], wc0[:, 0:1536])
        nc.sync.dma_start(wc_sb[:, 1536:4096], wc0[:, 1536:4096])
        nc.sync.dma_start(wc_sb[:, 4096:5632], wc0[:, 4096:5632])
        # [partition, chunk, ko, 512]; chunk 0 lives in wc_sb instead
        ET = cpool.tile([P, 4, KO, 512], bf16, tag="ET")
        nc.scalar.dma_start(ET[:, 1, :, :], et[1, :, :])
        nc.scalar.dma_start(ET[:, 2, :, :], et[2, :, :])
        mi_sb = cpool.tile([P, P + H], bf16, tag="mi")
        nc.sync.dma_start(mi_sb[:], mi[:])
        bias_sb = cpool.tile([P, 5], f32, tag="bias4")
        nc.sync.dma_start(bias_sb[:], bias4[:])
        nc.sync.dma_start(ET[:, 3, :, :], et[3, :, :])

        def w_ap(ko, a, b):      # weight cols a:b of ko-th 192-block
            return wc_sb[:, ko * 192 + a:ko * 192 + b]

        def et_ap(cc, ko):       # ET chunk cc, ko-th 512-col block
            if cc == 0:
                return wc_sb[:, 1536 + ko * 512:1536 + (ko + 1) * 512]
            return ET[:, cc, ko, :]

        bq_sb = bias_sb[:, 0:1]
        bk_sb = bias_sb[:, 1:2]
        g8_sb = bias_sb[:, 2:3]
        g12_sb = bias_sb[:, 3:4]
        bv_sb = bias_sb[:H, 4:5]
        tri_sb = mi_sb[:, 0:P]
        id_sb = mi_sb[:H, P:P + H]

        # Q^T and K^T both live on partitions 64:128 (matmul requires lhsT
        # and rhs to share a base partition; the packed [Wv|Wk] projection
        # puts K^T on PSUM rows 64:128 and DVE copies cannot shift rows).
        QT = cpool.tile([P, 1024], bf16, tag="QT")
        KT = cpool.tile([P, S], bf16, tag="KT")
        Vp = cpool.tile([P, NT, H + 1], bf16, tag="Vp")
        o_sb = cpool.tile([H + 1, 1024], f32, tag="osb")
        # HAM warmup scratch: memset FIRST on DVE so dependency-free N=512
        # matmuls start as early as possible and open the clock gate
        # (1.2 -> 2.4 GHz) before the weights DMA lands.
        wtile = cpool.tile([P, 512], bf16, tag="warm")
        nc.vector.memset(wtile[:], 0.0)
        nc.vector.memset(Vp[:, :, H:H + 1], 1.0)

        def vtranspose(vt, cc):
            for t in range(4):
                kt = cc * 4 + t
                pvt = psb.tile([P, H], bf16, tag="pj", name=f"pvt_{kt}")
                nc.tensor.transpose(
                    pvt[:], vt[:, t * P:(t + 1) * P], id_sb[:]
                )
                nc.vector.tensor_copy(Vp[:, kt, :H], pvt[:])

        vts = [None] * 4

        def pcopy(dst, src_ap, bias, eng):
            # PSUM->SBUF projection copy; with zero biases no copy reads
            # the bias DMA (early ones on the otherwise-idle ACT engine,
            # later ones on DVE), so scores never stall on it
            if zb:
                if eng == "act":
                    nc.scalar.activation(dst, src_ap, AF.Copy)
                else:
                    nc.vector.tensor_copy(dst, src_ap)
            else:
                nc.vector.tensor_scalar_add(dst, src_ap, bias)

        def vk_chunk(cc):
            # one pass of the ET chunk computes V^T (rows 0:64) + K^T (64:128)
            ps = psb.tile([P, 512], f32, tag="pj", name=f"vk_ps_{cc}")
            for ko in range(KO):
                nc.tensor.matmul(
                    ps[:], w_ap(ko, 0, 128), et_ap(cc, ko),
                    start=(ko == 0), stop=(ko == KO - 1),
                )
            eng = "act" if cc < 1 else "dve"
            pcopy(
                KT[H:P, cc * 512:(cc + 1) * 512], ps[H:P, :], bk_sb[H:P], eng
            )
            vt = vtp.tile([H, 512], bf16, tag="vt", name=f"vt_{cc}")
            pcopy(vt[:], ps[:H, :], bv_sb[:], eng)
            vts[cc] = vt

        def q_chunk(cc):
            # M=64 matmul targeting PSUM rows 64:128 so Q^T lands at base 64
            ps = psb.tile([P, 512], f32, tag="pj", name=f"q_ps_{cc}")
            for ko in range(KO):
                nc.tensor.matmul(
                    ps[H:P, :], w_ap(ko, 128, 192), et_ap(cc, ko),
                    start=(ko == 0), stop=(ko == KO - 1),
                )
            pcopy(
                QT[H:P, cc * 512:(cc + 1) * 512], ps[H:P, :], bq_sb[H:P],
                "act" if cc == 0 else "dve",
            )

        # --- attention ---
        outT0 = psb.tile([P, 512], f32, tag="os0", bufs=1)
        outT1 = psb.tile([P, 512], f32, tag="os1", bufs=1)

        # pvs[kt] = list of (outT, col0, rhs_ap) PV pieces for that key tile
        pvs = [None] * NT

        def tri_mult(pt, c0):
            # only the 128-col diagonal block needs masking; the rest of a
            # causal tail is all-ones
            nc.vector.tensor_tensor(
                pt[:, c0:c0 + P], pt[:, c0:c0 + P], tri_sb, ALU.mult
            )

        def sc(kt):
            # score regions trimmed to the causal need:
            #  kt 0..3  : cols [kt*128 : 1024] (slot0 tri tail + slot1 full)
            #  kt 4..7  : slot1 tri tail, cols [(kt-4)*128 : 512] of slot1
            #  kt 8..11 : both slots full; slot0 multiplied by 0/1 gate
            #  kt 12..15: slot1 full, exp-bias gated
            ps = psb.tile(
                [P, 1024], f32, tag="sc", name=f"sc_{kt}", bufs=2
            )
            kblk = KT[H:P, kt * P:(kt + 1) * P]
            pt = ptp.tile([P, 1024], bf16, tag="pt", name=f"pt_{kt}")
            if kt < 4 or (8 <= kt < 12):
                c0 = kt * P if kt < 4 else 0
                nc.tensor.matmul(
                    ps[:, c0:512], kblk, QT[H:P, c0:512],
                    start=True, stop=True, skip_group_check=True,
                )
                nc.tensor.matmul(
                    ps[:, 512:1024], kblk, QT[H:P, 512:1024],
                    start=True, stop=True, skip_group_check=True,
                )
                # two half-exps: slot0's PV can start while slot1 still exps;
                # for kt 8..11 the per-core 0/-30000 exp bias zeroes slot0 on
                # the core whose slot0 queries precede these keys
                if kt < 4:
                    nc.scalar.activation(pt[:, c0:512], ps[:, c0:512], AF.Exp)
                    tri_mult(pt, c0)
                else:
                    nc.scalar.activation(
                        pt[:, 0:512], ps[:, 0:512], AF.Exp, bias=g8_sb[:]
                    )
                nc.scalar.activation(
                    pt[:, 512:1024], ps[:, 512:1024], AF.Exp
                )
                pvs[kt] = [
                    (outT0, c0, pt[:, c0:512]),
                    (outT1, 0, pt[:, 512:1024]),
                ]
            else:
                c0 = (kt - 4) * P if kt < 12 else 0
                n = 512 - c0
                nc.tensor.matmul(
                    ps[:, 0:n], kblk, QT[H:P, 512 + c0:1024],
                    start=True, stop=True, skip_group_check=True,
                )
                if kt >= 12:
                    nc.scalar.activation(
                        pt[:, 0:n], ps[:, 0:n], AF.Exp, bias=g12_sb[:]
                    )
                else:
                    nc.scalar.activation(pt[:, 0:n], ps[:, 0:n], AF.Exp)
                    tri_mult(pt, 0)
                pvs[kt] = [(outT1, c0, pt[:, 0:n])]

        def pv(kt, stop0=False, stop1=False):
            for outT, c0, rhs in pvs[kt]:
                nc.tensor.matmul(
                    outT[:H + 1, c0:512], Vp[:, kt, :], rhs,
                    start=(kt == 0),
                    stop=(stop0 if outT is outT0 else stop1),
                    skip_group_check=True,
                )

        # --- emission order = per-engine FIFO order; hand-pipelined so PE
        # never waits on ACT/DVE and ACT starts exping early ---
        # HAM warmup: dependency-free N=512 matmuls on a zeroed scratch
        # tile run back-to-back from ~7.3us, opening the clock gate before
        # the first projection matmul (~11us). Results go to dead psum.
        for i in range(8):
            wps = psb.tile([P, 512], f32, tag="pj", name=f"warm_{i}")
            nc.tensor.matmul(
                wps[:], wtile[:, 0:P], wtile[:],
                start=True, stop=True, skip_group_check=True,
            )

        # kt 0 and 1 split in half-scores: the slot0 halves (which need
        # only Q chunk 0) issue before q_chunk(1), so ACT starts exping
        # ~2us earlier in the proj->attention transition
        eps = {}
        ept = {}

        def sc_half_a(kt):
            c0 = kt * P
            ps = psb.tile([P, 1024], f32, tag="sc", name=f"sc_{kt}", bufs=2)
            pt = ptp.tile([P, 1024], bf16, tag="pt", name=f"pt_{kt}")
            eps[kt], ept[kt] = ps, pt
            nc.tensor.matmul(
                ps[:, c0:512], KT[H:P, kt * P:(kt + 1) * P], QT[H:P, c0:512],
                start=True, stop=True, skip_group_check=True,
            )
            nc.scalar.activation(pt[:, c0:512], ps[:, c0:512], AF.Exp)
            tri_mult(pt, c0)

        def sc_half_b(kt):
            ps, pt = eps[kt], ept[kt]
            nc.tensor.matmul(
                ps[:, 512:1024], KT[H:P, kt * P:(kt + 1) * P],
                QT[H:P, 512:1024],
                start=True, stop=True, skip_group_check=True,
            )
            nc.scalar.activation(pt[:, 512:1024], ps[:, 512:1024], AF.Exp)
            pvs[kt] = [
                (outT0, kt * P, pt[:, kt * P:512]),
                (outT1, 0, pt[:, 512:1024]),
            ]

        def sc67():
            # kt 6 (256 cols) and 7 (128 cols) share one psum bank + exp
            ps = psb.tile([P, 1024], f32, tag="sc", name="sc_67", bufs=2)
            pt = ptp.tile([P, 1024], bf16, tag="pt", name="pt_67")
            nc.tensor.matmul(
                ps[:, 0:256], KT[H:P, 6 * P:7 * P], QT[H:P, 768:1024],
                start=True, stop=True, skip_group_check=True,
            )
            nc.tensor.matmul(
                ps[:, 256:384], KT[H:P, 7 * P:8 * P], QT[H:P, 896:1024],
                start=True, stop=True, skip_group_check=True,
            )
            nc.scalar.activation(pt[:, 0:384], ps[:, 0:384], AF.Exp)
            tri_mult(pt, 0)
            tri_mult(pt, 256)
            pvs[6] = [(outT1, 256, pt[:, 0:256])]
            pvs[7] = [(outT1, 384, pt[:, 256:384])]

        vk_chunk(0)
        q_chunk(0)
        sc_half_a(0)
        sc_half_a(1)
        q_chunk(1)
        sc_half_b(0)
        sc_half_b(1)
        sc(2)
        sc(3)
        vk_chunk(1)
        sc(4)
        sc(5)
        sc67()
        vk_chunk(2)
        sc(8)
        sc(9)
        vtranspose(vts[0], 0)
        vtranspose(vts[1], 1)
        pv(0)
        pv(1)
        pv(2)
        pv(3)
        pv(4)
        pv(5)
        vk_chunk(3)
        vtranspose(vts[2], 2)
        sc(10)
        pv(6)
        sc(11)
        pv(8)
        sc(12)
        pv(9)
        sc(13)
        vtranspose(vts[3], 3)
        sc(14)
        sc(15)
        pv(10)
        pv(11, stop0=True)
        nc.vector.tensor_copy(o_sb[:, 0:512], outT0[:H + 1, :])
        nc.sync.dma_start(out[:, 0:512], o_sb[:, 0:512])
        pv(12)
        pv(13)
        pv(14)
        pv(15)
        pv(7, stop1=True)
        nc.vector.tensor_copy(o_sb[:, 512:1024], outT1[:H + 1, :])
        nc.sync.dma_start(out[:, 512:1024], o_sb[:, 512:1024])

    nc.finalize()
    return nc


_CACHED = None


def _get_program(zb):
    global _CACHED
    if _CACHED is None or _CACHED[0] != zb:
        _CACHED = (zb, _build_program(zb))
    return _CACHED[1]


def _host_inputs(embeddings, Wq, bq, Wk, bk, Wv, bv):
    import ml_dtypes

    bf16 = ml_dtypes.bfloat16
    halves = _halves()
    # shared multiplicative tri diag mask: 1 where c >= k; plus identity
    tri = np.zeros((P, P), np.float32)
    for k in range(P):
        tri[k, k:] = 1.0
    ident = np.zeros((P, H), np.float32)
    ident[:H] = np.eye(H, dtype=np.float32)
    mi = np.ascontiguousarray(
        np.concatenate([tri, ident], axis=1)
    ).astype(bf16)

    def wlay(w):
        return np.asarray(w, np.float32).reshape(KO, P, H).transpose(1, 0, 2)

    wq8l = wlay(Wq) / 8.0
    wkl = wlay(Wk)
    wvl = wlay(Wv)
    wts = np.concatenate([wvl, wkl, wq8l], axis=2).reshape(P, 1536)
    bqf = np.asarray(bq, np.float32) / 8.0
    bkf = np.asarray(bk, np.float32)
    bvf = np.asarray(bv, np.float32)
    z64 = np.zeros(H, np.float32)
    bq8P = np.concatenate([z64, bqf])
    bkP = np.concatenate([z64, bkf])
    bvP = np.concatenate([bvf, z64])

    in_maps = []
    perms = []
    for c in range(8):
        b, h = c // 2, c % 2
        own = halves[h]
        other = halves[1 - h]
        rows = np.concatenate(
            [np.arange(a, z) for a, z in own] + [np.arange(a, z) for a, z in other]
        )
        perms.append(rows)
        ep = embeddings[b][rows]                      # [S, D] f32, permuted
        etl = np.ascontiguousarray(
            ep.T.reshape(KO, P, 4, 512).transpose(2, 1, 0, 3)
        ).astype(bf16).reshape(4, P, KO * 512)        # [cc, p, ko*512]
        g8v = np.full(P, 0.0 if h == 1 else NEG, np.float32)
        g12v = np.full(P, NEG if h == 1 else 0.0, np.float32)
        bias4 = np.ascontiguousarray(
            np.stack([bq8P, bkP, g8v, g12v, bvP], axis=1)
        )
        wc0l = np.ascontiguousarray(
            np.concatenate([wts, etl[0]], axis=1)
        ).astype(bf16)
        in_maps.append({
            "et": etl, "wc0": wc0l, "bias4": bias4, "mi": mi,
        })
    return in_maps, perms


def _run(embeddings, Wq, bq, Wk, bk, Wv, bv, trace=False):
    from concourse.bass_utils import run_bass_kernel_spmd

    zb = (
        not np.any(np.asarray(bq)) and not np.any(np.asarray(bk))
        and not np.any(np.asarray(bv))
    )
    nc = _get_program(zb)
    in_maps, perms = _host_inputs(embeddings, Wq, bq, Wk, bk, Wv, bv)
    res = run_bass_kernel_spmd(
        nc, in_maps, core_ids=list(range(8)), trace=trace,
        trace_cores=list(range(8)) if trace else None,
    )
    full = np.empty((B, S, H), np.float32)
    for c in range(8):
        b = c // 2
        o = res.results[c]["out"]                     # [65, 1024] f32
        full[b, perms[c][:1024]] = (o[:H] / o[H:H + 1]).T
    return full, res


def kernel(embeddings, Wq, bq, Wk, bk, Wv, bv):
    full, _ = _run(
        np.asarray(embeddings, np.float32), Wq, bq, Wk, bk, Wv, bv, trace=False
    )
    return full


# revision 14
# speedup vs baseline: 1.0818x; 1.0614x over previous
"""Causal single-head attention on 8 trn2 NeuronCores.

B=4, S=2048, D_MODEL=1024, D_HEAD=64, fp32 in/out.

Sharding: 2 cores per batch. Core half h=0 owns query tiles {0..3,12..15}
(rows 0:512, 1536:2048), h=1 owns {4..11} (rows 512:1536); both own 68
causal 128x128 blocks. The host feeds each core its batch's embeddings
already TRANSPOSED to E^T [dm, s] in bf16 with columns permuted so own
query rows come first - no on-device transposes/casts of E at all.

Per-core pipeline (identical SPMD program, all matmuls bf16):
  Warmup N=512 matmuls on a scratch tile open the PE HAM clock gate
  (1.2->2.4 GHz) while the first input DMA is in flight. Weights land in
  their own first dma_start so projections start ~2us earlier; the tri
  mask is a single shared 128x128 diagonal block (48KB with the identity,
  vs 540KB of per-tile tails - off-diagonal tail cols need no mask).
  Projections per 512-col chunk of E^T: one [Wv|Wk]-packed pass (V^T on
  PSUM rows 0:64, K^T on rows 64:128) plus, for the core's own 2 chunks,
  a Wq/8 pass targeting PSUM rows 64:128. Q^T and K^T both live on SBUF
  partitions 64:128 so score matmuls satisfy the shared-base-partition
  rule; V tiles are PE-transposed into Vp [128k, 16, 65] with a ones
  column (softmax denominator). With zero biases ALL PSUM->SBUF
  projection copies are bias-free (ACT engine early, DVE later), so no
  score matmul ever waits on the bias DMA.
  Attention over local key tiles kt, with score/exp/mask/PV regions
  trimmed to the causal need:
    kt 0..3  : cols [kt*128:1024] (slot0 tri tail + slot1 full), one exp
    kt 4..7  : slot1 tri tail only
    kt 8..11 : both slots; slot0 killed by a 0/-30000 exp bias on h=0
    kt 12..15: slot1 only; per-core 0/-30000 exp bias kills it on h=1
  PV accumulates out^T [65, 512] per slot in PSUM (col 64 = sum exp) and
  is DMA'd PSUM->HBM directly; the host does the final divide +
  transpose + scatter.
"""

import sys

if "/opt/trn_rl_repo" not in sys.path:
    sys.path.insert(0, "/opt/trn_rl_repo")

import numpy as np

B, S, D, H = 4, 2048, 1024, 64
P = 128
KO = D // P          # 8 dmodel chunks
NT = S // P          # 16 seq tiles
NEG = -30000.0


def _halves():
    return [[(0, 512), (1536, 2048)], [(512, 1536)]]


def _build_program(zb):
    import concourse.bacc as bacc
    import concourse.mybir as mybir
    import concourse.tile as tile

    f32 = mybir.dt.float32
    bf16 = mybir.dt.bfloat16
    AF = mybir.ActivationFunctionType
    ALU = mybir.AluOpType

    nc = bacc.Bacc()
    # et layout [chunk, partition, KO*512]: 8 KB contiguous per partition
    # per chunk -> big DMA descriptors (1 KB descriptors run ~21 GB/s/queue)
    et = nc.declare_dram_parameter("et", [4, P, KO * 512], bf16, isOutput=False)
    # weights + ET chunk 0: per partition cols 0:1536 = [Wv|Wk|Wq/8] x 8 ko
    # (192 each), cols 1536:5632 = chunk0
    wc0 = nc.declare_dram_parameter("wc0", [P, 1536 + 4096], bf16, isOutput=False)
    # cols: bq/8 | bk | g8 | g12n | bv (bv only rows 0:64 meaningful)
    bias4 = nc.declare_dram_parameter("bias4", [P, 5], f32, isOutput=False)
    # cols 0:128 = shared tri diag mask, cols 128:192 = identity (rows 0:64)
    mi = nc.declare_dram_parameter("mi", [P, P + H], bf16, isOutput=False)
    out = nc.declare_dram_parameter("out", [H + 1, 1024], f32, isOutput=True)

    from contextlib import ExitStack

    with tile.TileContext(nc) as tc, ExitStack() as ctx:
        cpool = ctx.enter_context(tc.tile_pool(name="const", bufs=1))
        vtp = ctx.enter_context(tc.tile_pool(name="vt", bufs=2))
        ptp = ctx.enter_context(tc.tile_pool(name="pt", bufs=10))
        psb = ctx.enter_context(tc.tile_pool(name="psb", bufs=2, space="PSUM"))

        # --- input DMAs: ONE hardware ring, strict need-order. Both DGE
        # rings share HBM bandwidth, so splitting streams across rings only
        # starves the critical weights transfer; ordering on one ring gives
        # strict priority.
        wc_sb = cpool.tile([P, 1536 + 4096], bf16, tag="wc0")
        nc.sync.dma_start(wc_sb[:, 0:1536], wc0[:, 0:1536])
        nc.sync.dma_start(wc_sb[:, 1536:4096], wc0[:, 1536:4096])
        nc.sync.dma_start(wc_sb[:, 4096:5632], wc0[:, 4096:5632])
        mi_sb = cpool.tile([P, P + H], bf16, tag="mi")
        nc.sync.dma_start(mi_sb[:], mi[:])
        bias_sb = cpool.tile([P, 5], f32, tag="bias4")
        nc.sync.dma_start(bias_sb[:], bias4[:])
        # [partition, chunk, ko, 512]; chunk 0 lives in wc_sb instead
        ET = cpool.tile([P, 4, KO, 512], bf16, tag="ET")
        nc.sync.dma_start(ET[:, 1, :, :], et[1, :, :])
        nc.sync.dma_start(ET[:, 2, :, :], et[2, :, :])
        nc.sync.dma_start(ET[:, 3, :, :], et[3, :, :])

        def w_ap(ko, a, b):      # weight cols a:b of ko-th 192-block
            return wc_sb[:, ko * 192 + a:ko * 192 + b]

        def et_ap(cc, ko):       # ET chunk cc, ko-th 512-col block
            if cc == 0:
                return wc_sb[:, 1536 + ko * 512:1536 + (ko + 1) * 512]
            return ET[:, cc, ko, :]

        bq_sb = bias_sb[:, 0:1]
        bk_sb = bias_sb[:, 1:2]
        g8_sb = bias_sb[:, 2:3]
        g12_sb = bias_sb[:, 3:4]
        bv_sb = bias_sb[:H, 4:5]
        tri_sb = mi_sb[:, 0:P]
        id_sb = mi_sb[:H, P:P + H]

        # Q^T and K^T both live on partitions 64:128 (matmul requires lhsT
        # and rhs to share a base partition; the packed [Wv|Wk] projection
        # puts K^T on PSUM rows 64:128 and DVE copies cannot shift rows).
        QT = cpool.tile([P, 1024], bf16, tag="QT")
        KT = cpool.tile([P, S], bf16, tag="KT")
        Vp = cpool.tile([P, NT, H + 1], bf16, tag="Vp")
        o_sb = cpool.tile([H + 1, 1024], f32, tag="osb")
        # HAM warmup scratch: memset FIRST on DVE so dependency-free N=512
        # matmuls start as early as possible and open the clock gate
        # (1.2 -> 2.4 GHz) before the weights DMA lands.
        wtile = cpool.tile([P, 512], bf16, tag="warm")
        nc.vector.memset(wtile[:], 0.0)
        nc.vector.memset(Vp[:, :, H:H + 1], 1.0)

        def vtranspose(vt, cc):
            for t in range(4):
                kt = cc * 4 + t
                pvt = psb.tile([P, H], bf16, tag="pj", name=f"pvt_{kt}")
                nc.tensor.transpose(
                    pvt[:], vt[:, t * P:(t + 1) * P], id_sb[:]
                )
                nc.vector.tensor_copy(Vp[:, kt, :H], pvt[:])

        vts = [None] * 4

        def pcopy(dst, src_ap, bias, eng):
            # PSUM->SBUF projection copy; with zero biases no copy reads
            # the bias DMA (early ones on the otherwise-idle ACT engine,
            # later ones on DVE), so scores never stall on it
            if zb:
                if eng == "act":
                    nc.scalar.activation(dst, src_ap, AF.Copy)
                else:
                    nc.vector.tensor_copy(dst, src_ap)
            else:
                nc.vector.tensor_scalar_add(dst, src_ap, bias)

        def vk_chunk(cc):
            # one pass of the ET chunk computes V^T (rows 0:64) + K^T (64:128)
            ps = psb.tile([P, 512], f32, tag="pj", name=f"vk_ps_{cc}")
            for ko in range(KO):
                nc.tensor.matmul(
                    ps[:], w_ap(ko, 0, 128), et_ap(cc, ko),
                    start=(ko == 0), stop=(ko == KO - 1),
                )
            eng = "act" if cc < 1 else "dve"
            pcopy(
                KT[H:P, cc * 512:(cc + 1) * 512], ps[H:P, :], bk_sb[H:P], eng
            )
            vt = vtp.tile([H, 512], bf16, tag="vt", name=f"vt_{cc}")
            pcopy(vt[:], ps[:H, :], bv_sb[:], eng)
            vts[cc] = vt

        def q_chunk(cc):
            # M=64 matmul targeting PSUM rows 64:128 so Q^T lands at base 64
            ps = psb.tile([P, 512], f32, tag="pj", name=f"q_ps_{cc}")
            for ko in range(KO):
                nc.tensor.matmul(
                    ps[H:P, :], w_ap(ko, 128, 192), et_ap(cc, ko),
                    start=(ko == 0), stop=(ko == KO - 1),
                )
            pcopy(
                QT[H:P, cc * 512:(cc + 1) * 512], ps[H:P, :], bq_sb[H:P],
                "act" if cc == 0 else "dve",
            )

        # --- attention ---
        outT0 = psb.tile([P, 512], f32, tag="os0", bufs=1)
        outT1 = psb.tile([P, 512], f32, tag="os1", bufs=1)

        # pvs[kt] = list of (outT, col0, rhs_ap) PV pieces for that key tile
        pvs = [None] * NT

        def tri_mult(pt, c0):
            # only the 128-col diagonal block needs masking; the rest of a
            # causal tail is all-ones
            nc.vector.tensor_tensor(
                pt[:, c0:c0 + P], pt[:, c0:c0 + P], tri_sb, ALU.mult
            )

        def sc(kt):
            # score regions trimmed to the causal need:
            #  kt 0..3  : cols [kt*128 : 1024] (slot0 tri tail + slot1 full)
            #  kt 4..7  : slot1 tri tail, cols [(kt-4)*128 : 512] of slot1
            #  kt 8..11 : both slots full; slot0 multiplied by 0/1 gate
            #  kt 12..15: slot1 full, exp-bias gated
            ps = psb.tile(
                [P, 1024], f32, tag="sc", name=f"sc_{kt}", bufs=2
            )
            kblk = KT[H:P, kt * P:(kt + 1) * P]
            pt = ptp.tile([P, 1024], bf16, tag="pt", name=f"pt_{kt}")
            if kt < 4 or (8 <= kt < 12):
                c0 = kt * P if kt < 4 else 0
                nc.tensor.matmul(
                    ps[:, c0:512], kblk, QT[H:P, c0:512],
                    start=True, stop=True, skip_group_check=True,
                )
                nc.tensor.matmul(
                    ps[:, 512:1024], kblk, QT[H:P, 512:1024],
                    start=True, stop=True, skip_group_check=True,
                )
                # two half-exps: slot0's PV can start while slot1 still exps;
                # for kt 8..11 the per-core 0/-30000 exp bias zeroes slot0 on
                # the core whose slot0 queries precede these keys
                if kt < 4:
                    nc.scalar.activation(pt[:, c0:512], ps[:, c0:512], AF.Exp)
                    tri_mult(pt, c0)
                else:
                    nc.scalar.activation(
                        pt[:, 0:512], ps[:, 0:512], AF.Exp, bias=g8_sb[:]
                    )
                nc.scalar.activation(
                    pt[:, 512:1024], ps[:, 512:1024], AF.Exp
                )
                pvs[kt] = [
                    (outT0, c0, pt[:, c0:512]),
                    (outT1, 0, pt[:, 512:1024]),
                ]
            else:
                c0 = (kt - 4) * P if kt < 12 else 0
                n = 512 - c0
                nc.tensor.matmul(
                    ps[:, 0:n], kblk, QT[H:P, 512 + c0:1024],
                    start=True, stop=True, skip_group_check=True,
                )
                if kt >= 12:
                    nc.scalar.activation(
                        pt[:, 0:n], ps[:, 0:n], AF.Exp, bias=g12_sb[:]
                    )
                else:
                    nc.scalar.activation(pt[:, 0:n], ps[:, 0:n], AF.Exp)
                    tri_mult(pt, 0)
                pvs[kt] = [(outT1, c0, pt[:, 0:n])]

        def pv(kt, stop0=False, stop1=False):
            for outT, c0, rhs in pvs[kt]:
                nc.tensor.matmul(
                    outT[:H + 1, c0:512], Vp[:, kt, :], rhs,
                    start=(kt == 0),
                    stop=(stop0 if outT is outT0 else stop1),
                    skip_group_check=True,
                )

        # --- emission order = per-engine FIFO order; hand-pipelined so PE
        # never waits on ACT/DVE and ACT starts exping early ---
        # HAM warmup: dependency-free N=512 matmuls on a zeroed scratch
        # tile run back-to-back from ~7.3us, opening the clock gate before
        # the first projection matmul (~11us). Results go to dead psum.
        for i in range(8):
            wps = psb.tile([P, 512], f32, tag="pj", name=f"warm_{i}")
            nc.tensor.matmul(
                wps[:], wtile[:, 0:P], wtile[:],
                start=True, stop=True, skip_group_check=True,
            )

        # kt 0 and 1 split in half-scores: the slot0 halves (which need
        # only Q chunk 0) issue before q_chunk(1), so ACT starts exping
        # ~2us earlier in the proj->attention transition
        eps = {}
        ept = {}

        def sc_half_a(kt):
            c0 = kt * P
            ps = psb.tile([P, 1024], f32, tag="sc", name=f"sc_{kt}", bufs=2)
            pt = ptp.tile([P, 1024], bf16, tag="pt", name=f"pt_{kt}")
            eps[kt], ept[kt] = ps, pt
            nc.tensor.matmul(
                ps[:, c0:512], KT[H:P, kt * P:(kt + 1) * P], QT[H:P, c0:512],
                start=True, stop=True, skip_group_check=True,
            )
            nc.scalar.activation(pt[:, c0:512], ps[:, c0:512], AF.Exp)
            tri_mult(pt, c0)

        def sc_half_b(kt):
            ps, pt = eps[kt], ept[kt]
            nc.tensor.matmul(
                ps[:, 512:1024], KT[H:P, kt * P:(kt + 1) * P],
                QT[H:P, 512:1024],
                start=True, stop=True, skip_group_check=True,
            )
            nc.scalar.activation(pt[:, 512:1024], ps[:, 512:1024], AF.Exp)
            pvs[kt] = [
                (outT0, kt * P, pt[:, kt * P:512]),
                (outT1, 0, pt[:, 512:1024]),
            ]

        def sc67():
            # kt 6 (256 cols) and 7 (128 cols) share one psum bank + exp
            ps = psb.tile([P, 1024], f32, tag="sc", name="sc_67", bufs=2)
            pt = ptp.tile([P, 1024], bf16, tag="pt", name="pt_67")
            nc.tensor.matmul(
                ps[:, 0:256], KT[H:P, 6 * P:7 * P], QT[H:P, 768:1024],
                start=True, stop=True, skip_group_check=True,
            )
            nc.tensor.matmul(
                ps[:, 256:384], KT[H:P, 7 * P:8 * P], QT[H:P, 896:1024],
                start=True, stop=True, skip_group_check=True,
            )
            nc.scalar.activation(pt[:, 0:384], ps[:, 0:384], AF.Exp)
            tri_mult(pt, 0)
            tri_mult(pt, 256)
            pvs[6] = [(outT1, 256, pt[:, 0:256])]
            pvs[7] = [(outT1, 384, pt[:, 256:384])]

        vk_chunk(0)
        q_chunk(0)
        sc_half_a(0)
        sc_half_a(1)
        q_chunk(1)
        sc_half_b(0)
        sc_half_b(1)
        sc(2)
        sc(3)
        vk_chunk(1)
        sc(4)
        sc(5)
        sc67()
        vk_chunk(2)
        sc(8)
        sc(9)
        vtranspose(vts[0], 0)
        vtranspose(vts[1], 1)
        pv(0)
        pv(1)
        pv(2)
        pv(3)
        pv(4)
        pv(5)
        vk_chunk(3)
        vtranspose(vts[2], 2)
        sc(10)
        pv(6)
        sc(11)
        pv(8)
        sc(12)
        pv(9)
        sc(13)
        vtranspose(vts[3], 3)
        sc(14)
        sc(15)
        pv(10)
        pv(11, stop0=True)
        nc.vector.tensor_copy(o_sb[:, 0:512], outT0[:H + 1, :])
        nc.sync.dma_start(out[:, 0:512], o_sb[:, 0:512])
        pv(12)
        pv(13)
        pv(14)
        pv(15)
        pv(7, stop1=True)
        nc.vector.tensor_copy(o_sb[:, 512:1024], outT1[:H + 1, :])
        nc.sync.dma_start(out[:, 512:1024], o_sb[:, 512:1024])

    nc.finalize()
    return nc


_CACHED = None


def _get_program(zb):
    global _CACHED
    if _CACHED is None or _CACHED[0] != zb:
        _CACHED = (zb, _build_program(zb))
    return _CACHED[1]


def _host_inputs(embeddings, Wq, bq, Wk, bk, Wv, bv):
    import ml_dtypes

    bf16 = ml_dtypes.bfloat16
    halves = _halves()
    # shared multiplicative tri diag mask: 1 where c >= k; plus identity
    tri = np.zeros((P, P), np.float32)
    for k in range(P):
        tri[k, k:] = 1.0
    ident = np.zeros((P, H), np.float32)
    ident[:H] = np.eye(H, dtype=np.float32)
    mi = np.ascontiguousarray(
        np.concatenate([tri, ident], axis=1)
    ).astype(bf16)

    def wlay(w):
        return np.asarray(w, np.float32).reshape(KO, P, H).transpose(1, 0, 2)

    wq8l = wlay(Wq) / 8.0
    wkl = wlay(Wk)
    wvl = wlay(Wv)
    wts = np.concatenate([wvl, wkl, wq8l], axis=2).reshape(P, 1536)
    bqf = np.asarray(bq, np.float32) / 8.0
    bkf = np.asarray(bk, np.float32)
    bvf = np.asarray(bv, np.float32)
    z64 = np.zeros(H, np.float32)
    bq8P = np.concatenate([z64, bqf])
    bkP = np.concatenate([z64, bkf])
    bvP = np.concatenate([bvf, z64])

    in_maps = []
    perms = []
    for c in range(8):
        b, h = c // 2, c % 2
        own = halves[h]
        other = halves[1 - h]
        rows = np.concatenate(
            [np.arange(a, z) for a, z in own] + [np.arange(a, z) for a, z in other]
        )
        perms.append(rows)
        ep = embeddings[b][rows]                      # [S, D] f32, permuted
        etl = np.ascontiguousarray(
            ep.T.reshape(KO, P, 4, 512).transpose(2, 1, 0, 3)
        ).astype(bf16).reshape(4, P, KO * 512)        # [cc, p, ko*512]
        g8v = np.full(P, 0.0 if h == 1 else NEG, np.float32)
        g12v = np.full(P, NEG if h == 1 else 0.0, np.float32)
        bias4 = np.ascontiguousarray(
            np.stack([bq8P, bkP, g8v, g12v, bvP], axis=1)
        )
        wc0l = np.ascontiguousarray(
            np.concatenate([wts, etl[0]], axis=1)
        ).astype(bf16)
        in_maps.append({
            "et": etl, "wc0": wc0l, "bias4": bias4, "mi": mi,
        })
    return in_maps, perms


def _run(embeddings, Wq, bq, Wk, bk, Wv, bv, trace=False):
    from concourse.bass_utils import run_bass_kernel_spmd

    zb = (
        not np.any(np.asarray(bq)) and not np.any(np.asarray(bk))
        and not np.any(np.asarray(bv))
    )
    nc = _get_program(zb)
    in_maps, perms = _host_inputs(embeddings, Wq, bq, Wk, bk, Wv, bv)
    res = run_bass_kernel_spmd(
        nc, in_maps, core_ids=list(range(8)), trace=trace,
        trace_cores=list(range(8)) if trace else None,
    )
    full = np.empty((B, S, H), np.float32)
    for c in range(8):
        b = c // 2
        o = res.results[c]["out"]                     # [65, 1024] f32
        full[b, perms[c][:1024]] = (o[:H] / o[H:H + 1]).T
    return full, res


def kernel(embeddings, Wq, bq, Wk, bk, Wv, bv):
    full, _ = _run(
        np.asarray(embeddings, np.float32), Wq, bq, Wk, bk, Wv, bv, trace=False
    )
    return full


# revision 18
# speedup vs baseline: 1.0872x; 1.0050x over previous
"""Causal single-head attention on 8 trn2 NeuronCores - split-72 geometry.

B=4, S=2048, D_MODEL=1024, D_HEAD=64, fp32 in/out.

Sharding: 2 cores per batch with an interleaved query-tile split
(h=0 owns tiles {0,2,4,6,9,11,13,15}, h=1 the complement; 68 causal
128x128 blocks each). The host feeds each core E^T [dm, s] bf16 with
columns ordered [own tiles DESCENDING | other tiles ascending]. With
own-descending query columns, the queries needing key tile at position
p form a PREFIX of the 1024 QT columns, so each score unit computes a
prefix range:
  position p 0..7  (own keys):   width (p+1)*128, diag tri at last block
  position p 8..15 (other keys): width (16-p)*128, last block either
    fully causal or fully dead - killed by a per-core 0/-30000 exp bias
Total 72 blocks/core vs 84 for the contiguous-half split (68 = ideal).

Per-core pipeline (identical SPMD program, all matmuls bf16):
  Warmup N=512 matmuls open the PE HAM clock gate while the first input
  DMA is in flight; inputs stream over BOTH hardware DGE rings (Sync +
  Act). Projections per 512-col chunk of E^T: one [Wv|Wk]-packed pass
  (V^T on PSUM rows 0:64, K^T on 64:128) plus, for the core's own 2
  chunks, a Wq/8 pass targeting PSUM rows 64:128. Q^T/K^T live on SBUF
  partitions 64:128 (shared-base-partition rule); V tiles are
  PE-transposed into Vp [128k, 16, 65] with a ones column (softmax
  denominator). Zero biases -> all projection PSUM->SBUF copies are
  bias-free.
  PV accumulates out^T [65, 1024] in one 2-bank PSUM tile; start=True
  resets a whole 512-col psum bank, so the widest unit of each bank
  (3, 7) is emitted first and opens its bank with one full-bank start,
  everything else accumulates. Output drains in 3 pieces as column
  regions complete; the host does the final divide+transpose+scatter.
"""

import sys

if "/opt/trn_rl_repo" not in sys.path:
    sys.path.insert(0, "/opt/trn_rl_repo")

import numpy as np

B, S, D, H = 4, 2048, 1024, 64
P = 128
KO = D // P          # 8 dmodel chunks
NT = S // P          # 16 seq tiles
NEG = -30000.0
OWN0 = [0, 2, 4, 6, 9, 11, 13, 15]   # h=0 query tiles
OWN1 = [t for t in range(16) if t not in OWN0]


def _order(h):
    own = OWN0 if h == 0 else OWN1
    other = OWN1 if h == 0 else OWN0
    return sorted(own, reverse=True) + sorted(other)


def _width(p):
    return p + 1 if p < 8 else 16 - p


def _build_program(zb):
    import concourse.bacc as bacc
    import concourse.mybir as mybir
    import concourse.tile as tile

    f32 = mybir.dt.float32
    bf16 = mybir.dt.bfloat16
    AF = mybir.ActivationFunctionType
    ALU = mybir.AluOpType

    nc = bacc.Bacc()
    et = nc.declare_dram_parameter("et", [4, P, KO * 512], bf16, isOutput=False)
    # per partition cols 0:1536 = [Wv|Wk|Wq/8] x 8 ko, cols 1536:5632 = chunk0
    wc0 = nc.declare_dram_parameter("wc0", [P, 1536 + 4096], bf16, isOutput=False)
    # cols: bq/8 | bk | bv | bg[8..15] (0 or NEG per core)
    biasg = nc.declare_dram_parameter("biasg", [P, 11], f32, isOutput=False)
    # cols 0:128 = shared tri diag mask, cols 128:192 = identity (rows 0:64)
    mi = nc.declare_dram_parameter("mi", [P, P + H], bf16, isOutput=False)
    out = nc.declare_dram_parameter("out", [H + 1, 1024], f32, isOutput=True)

    from contextlib import ExitStack

    with tile.TileContext(nc) as tc, ExitStack() as ctx:
        cpool = ctx.enter_context(tc.tile_pool(name="const", bufs=1))
        vtp = ctx.enter_context(tc.tile_pool(name="vt", bufs=2))
        ptp = ctx.enter_context(tc.tile_pool(name="pt", bufs=10))
        psb = ctx.enter_context(tc.tile_pool(name="psb", bufs=2, space="PSUM"))

        # --- input DMAs: ONE hardware ring, strict need-order (both DGE
        # rings share HBM bandwidth; splitting starves the critical pieces)
        wc_sb = cpool.tile([P, 1536 + 4096], bf16, tag="wc0")
        nc.sync.dma_start(wc_sb[:, 0:1536], wc0[:, 0:1536])
        nc.sync.dma_start(wc_sb[:, 1536:4096], wc0[:, 1536:4096])
        nc.sync.dma_start(wc_sb[:, 4096:5632], wc0[:, 4096:5632])
        mi_sb = cpool.tile([P, P + H], bf16, tag="mi")
        nc.sync.dma_start(mi_sb[:], mi[:])
        bias_sb = cpool.tile([P, 11], f32, tag="biasg")
        nc.sync.dma_start(bias_sb[:], biasg[:])
        ET = cpool.tile([P, 4, KO, 512], bf16, tag="ET")
        nc.sync.dma_start(ET[:, 1, :, :], et[1, :, :])
        nc.sync.dma_start(ET[:, 2, :, :], et[2, :, :])
        nc.sync.dma_start(ET[:, 3, :, :], et[3, :, :])

        def w_ap(ko, a, b):
            return wc_sb[:, ko * 192 + a:ko * 192 + b]

        def et_ap(cc, ko):
            if cc == 0:
                return wc_sb[:, 1536 + ko * 512:1536 + (ko + 1) * 512]
            return ET[:, cc, ko, :]

        bq_sb = bias_sb[:, 0:1]
        bk_sb = bias_sb[:, 1:2]
        bv_sb = bias_sb[:H, 2:3]

        def bg_sb(p):
            return bias_sb[:, 3 + (p - 8):4 + (p - 8)]

        tri_sb = mi_sb[:, 0:P]
        id_sb = mi_sb[:H, P:P + H]

        QT = cpool.tile([P, 1024], bf16, tag="QT")
        KT = cpool.tile([P, S], bf16, tag="KT")
        Vp = cpool.tile([P, NT, H + 1], bf16, tag="Vp")
        o_sb = cpool.tile([H + 1, 1024], f32, tag="osb")
        wtile = cpool.tile([P, 512], bf16, tag="warm")
        nc.vector.memset(wtile[:], 0.0)
        nc.vector.memset(Vp[:, :, H:H + 1], 1.0)

        def vtranspose(vt, cc):
            for t in range(4):
                kt = cc * 4 + t
                pvt = psb.tile([P, H], bf16, tag="pj", name=f"pvt_{kt}")
                nc.tensor.transpose(
                    pvt[:], vt[:, t * P:(t + 1) * P], id_sb[:]
                )
                nc.vector.tensor_copy(Vp[:, kt, :H], pvt[:])

        vts = [None] * 4

        def pcopy(dst, src_ap, bias, eng):
            if zb:
                if eng == "act":
                    nc.scalar.activation(dst, src_ap, AF.Copy)
                else:
                    nc.vector.tensor_copy(dst, src_ap)
            else:
                nc.vector.tensor_scalar_add(dst, src_ap, bias)

        def vk_chunk(cc):
            ps = psb.tile([P, 512], f32, tag="pj", name=f"vk_ps_{cc}")
            for ko in range(KO):
                nc.tensor.matmul(
                    ps[:], w_ap(ko, 0, 128), et_ap(cc, ko),
                    start=(ko == 0), stop=(ko == KO - 1),
                )
            eng = "act" if cc < 1 else "dve"
            pcopy(
                KT[H:P, cc * 512:(cc + 1) * 512], ps[H:P, :], bk_sb[H:P], eng
            )
            vt = vtp.tile([H, 512], bf16, tag="vt", name=f"vt_{cc}")
            pcopy(vt[:], ps[:H, :], bv_sb[:], eng)
            vts[cc] = vt

        def q_chunk(cc):
            ps = psb.tile([P, 512], f32, tag="pj", name=f"q_ps_{cc}")
            for ko in range(KO):
                nc.tensor.matmul(
                    ps[H:P, :], w_ap(ko, 128, 192), et_ap(cc, ko),
                    start=(ko == 0), stop=(ko == KO - 1),
                )
            pcopy(
                QT[H:P, cc * 512:(cc + 1) * 512], ps[H:P, :], bq_sb[H:P],
                "act" if cc == 0 else "dve",
            )

        # --- attention: 16 prefix-range units over one 2-bank out^T psum
        outT = psb.tile([P, 1024], f32, tag="os", bufs=1)
        pts = [None] * NT

        def col_pieces(w128, bound=512):
            # split [0, w128) at the 512-col psum bank boundary
            if w128 <= bound:
                return [(0, w128)]
            return [(0, bound), (bound, w128)]

        def scores(p):
            w = _width(p) * P
            ps = psb.tile([P, 1024], f32, tag="sc", name=f"sc_{p}", bufs=2)
            pt = ptp.tile([P, 1024], bf16, tag="pt", name=f"pt_{p}")
            pts[p] = pt
            kblk = KT[H:P, p * P:(p + 1) * P]
            for a, b in col_pieces(w):
                nc.tensor.matmul(
                    ps[:, a:b], kblk, QT[H:P, a:b],
                    start=True, stop=True, skip_group_check=True,
                )
            if p < 8:
                # own key: exp all, tri-mask the diagonal (last) block
                for a, b in col_pieces(w):
                    nc.scalar.activation(pt[:, a:b], ps[:, a:b], AF.Exp)
                nc.vector.tensor_tensor(
                    pt[:, w - P:w], pt[:, w - P:w], tri_sb, ALU.mult
                )
            else:
                # other key: last block fully causal or fully dead
                # (0/-30000 per-core exp bias)
                if w > P:
                    for a, b in col_pieces(w - P):
                        nc.scalar.activation(pt[:, a:b], ps[:, a:b], AF.Exp)
                nc.scalar.activation(
                    pt[:, w - P:w], ps[:, w - P:w], AF.Exp, bias=bg_sb(p)
                )

        def pv(p, stops=()):
            # start=True resets the ENTIRE 512-col psum bank, so each bank
            # gets exactly one start: unit 3 opens bank A with its full
            # [0:512] write, unit 7 opens bank B with [512:1024]; they are
            # emitted before any other writer of their bank.
            w = _width(p) * P
            pt = pts[p]
            if p == 3:
                pieces = [(0, 512, True)]
            elif p == 7:
                pieces = [(0, 512, False), (512, 1024, True)]
            else:
                pieces = [(a, b, False) for a, b in col_pieces(w)]
            for a, b, st in pieces:
                nc.tensor.matmul(
                    outT[:H + 1, a:b], Vp[:, p, :], pt[:, a:b],
                    start=st, stop=(a in stops),
                    skip_group_check=True,
                )

        def drain(a, b):
            nc.vector.tensor_copy(o_sb[:, a:b], outT[:H + 1, a:b])
            nc.sync.dma_start(out[:, a:b], o_sb[:, a:b])

        # --- emission order = per-engine FIFO order ---
        for i in range(8):
            wps = psb.tile([P, 512], f32, tag="pj", name=f"warm_{i}")
            nc.tensor.matmul(
                wps[:], wtile[:, 0:P], wtile[:],
                start=True, stop=True, skip_group_check=True,
            )

        # transposes and ready pvs are placed to fill the PE bubble while
        # each chunk's PSUM->SBUF copies (ACT/DVE) land
        vk_chunk(0)
        q_chunk(0)
        vtranspose(vts[0], 0)
        scores(3)
        scores(0)
        pv(3)
        scores(1)
        pv(0)
        scores(2)
        pv(1)
        pv(2)
        vk_chunk(1)
        q_chunk(1)
        vtranspose(vts[1], 1)
        scores(7)
        scores(4)
        pv(7)
        scores(5)
        pv(4)
        scores(6)
        pv(5)
        pv(6)
        vk_chunk(2)
        vtranspose(vts[2], 2)
        scores(8)
        scores(9)
        pv(8)
        scores(10)
        pv(9)
        drain(768, 1024)
        scores(11)
        pv(10)
        vk_chunk(3)
        pv(11, stops=(512,))
        vtranspose(vts[3], 3)
        scores(12)
        scores(13)
        pv(12)
        scores(14)
        pv(13)
        drain(256, 768)
        scores(15)
        pv(14)
        pv(15, stops=(0,))
        drain(0, 256)

    nc.finalize()
    return nc


_CACHED = None


def _get_program(zb):
    global _CACHED
    if _CACHED is None or _CACHED[0] != zb:
        _CACHED = (zb, _build_program(zb))
    return _CACHED[1]


def _host_inputs(embeddings, Wq, bq, Wk, bk, Wv, bv):
    import ml_dtypes

    bf16 = ml_dtypes.bfloat16
    tri = np.zeros((P, P), np.float32)
    for k in range(P):
        tri[k, k:] = 1.0
    ident = np.zeros((P, H), np.float32)
    ident[:H] = np.eye(H, dtype=np.float32)
    mi = np.ascontiguousarray(
        np.concatenate([tri, ident], axis=1)
    ).astype(bf16)

    def wlay(w):
        return np.asarray(w, np.float32).reshape(KO, P, H).transpose(1, 0, 2)

    wq8l = wlay(Wq) / 8.0
    wkl = wlay(Wk)
    wvl = wlay(Wv)
    wts = np.concatenate([wvl, wkl, wq8l], axis=2).reshape(P, 1536)
    bqf = np.asarray(bq, np.float32) / 8.0
    bkf = np.asarray(bk, np.float32)
    bvf = np.asarray(bv, np.float32)
    z64 = np.zeros(H, np.float32)
    bq8P = np.concatenate([z64, bqf])
    bkP = np.concatenate([z64, bkf])
    bvP = np.concatenate([bvf, z64])

    in_maps = []
    perms = []
    for c in range(8):
        b, h = c // 2, c % 2
        order = _order(h)
        own = set(OWN0 if h == 0 else OWN1)
        rows = np.concatenate(
            [np.arange(t * P, (t + 1) * P) for t in order]
        )
        perms.append(rows)
        ep = embeddings[b][rows]                      # [S, D] f32, permuted
        etl = np.ascontiguousarray(
            ep.T.reshape(KO, P, 4, 512).transpose(2, 1, 0, 3)
        ).astype(bf16).reshape(4, P, KO * 512)        # [cc, p, ko*512]
        # bg[p]: 0 if the last block of unit p is fully causal, NEG if dead
        bgs = []
        for p in range(8, 16):
            key = order[p]
            s = sum(1 for t in own if t >= key)
            bgs.append(
                np.full(P, 0.0 if s == _width(p) else NEG, np.float32)
            )
        biasg = np.ascontiguousarray(
            np.stack([bq8P, bkP, bvP] + bgs, axis=1)
        )
        wc0l = np.ascontiguousarray(
            np.concatenate([wts, etl[0]], axis=1)
        ).astype(bf16)
        in_maps.append({
            "et": etl, "wc0": wc0l, "biasg": biasg, "mi": mi,
        })
    return in_maps, perms


def _run(embeddings, Wq, bq, Wk, bk, Wv, bv, trace=False):
    from concourse.bass_utils import run_bass_kernel_spmd

    zb = (
        not np.any(np.asarray(bq)) and not np.any(np.asarray(bk))
        and not np.any(np.asarray(bv))
    )
    nc = _get_program(zb)
    in_maps, perms = _host_inputs(embeddings, Wq, bq, Wk, bk, Wv, bv)
    res = run_bass_kernel_spmd(
        nc, in_maps, core_ids=list(range(8)), trace=trace,
        trace_cores=list(range(8)) if trace else None,
    )
    full = np.empty((B, S, H), np.float32)
    for c in range(8):
        b = c // 2
        o = res.results[c]["out"]                     # [65, 1024] f32
        full[b, perms[c][:1024]] = (o[:H] / o[H:H + 1]).T
    return full, res


def kernel(embeddings, Wq, bq, Wk, bk, Wv, bv):
    full, _ = _run(
        np.asarray(embeddings, np.float32), Wq, bq, Wk, bk, Wv, bv, trace=False
    )
    return full


# revision 19
# speedup vs baseline: 1.1253x; 1.0350x over previous
"""Causal single-head attention on 8 trn2 NeuronCores - split-72 geometry.

B=4, S=2048, D_MODEL=1024, D_HEAD=64, fp32 in/out.

Sharding: 2 cores per batch with an interleaved query-tile split
(h=0 owns tiles {0,2,4,6,9,11,13,15}, h=1 the complement; 68 causal
128x128 blocks each). The host feeds each core E^T [dm, s] bf16 with
columns ordered [own tiles DESCENDING | other tiles ascending]. With
own-descending query columns, the queries needing key tile at position
p form a PREFIX of the 1024 QT columns, so each score unit computes a
prefix range:
  position p 0..7  (own keys):   width (p+1)*128, diag tri at last block
  position p 8..15 (other keys): width (16-p)*128, last block either
    fully causal or fully dead - killed by a per-core 0/-30000 exp bias
Total 72 blocks/core vs 84 for the contiguous-half split (68 = ideal).

Per-core pipeline (identical SPMD program, all matmuls bf16):
  Warmup N=512 matmuls open the PE HAM clock gate while the first input
  DMA is in flight; inputs stream over BOTH hardware DGE rings (Sync +
  Act). Projections per 512-col chunk of E^T: one [Wv|Wk]-packed pass
  (V^T on PSUM rows 0:64, K^T on 64:128) plus, for the core's own 2
  chunks, a Wq/8 pass targeting PSUM rows 64:128. Q^T/K^T live on SBUF
  partitions 64:128 (shared-base-partition rule); V tiles are
  PE-transposed into Vp [128k, 16, 65] with a ones column (softmax
  denominator). Zero biases -> all projection PSUM->SBUF copies are
  bias-free.
  PV accumulates out^T [65, 1024] in one 2-bank PSUM tile; start=True
  resets a whole 512-col psum bank, so the widest unit of each bank
  (3, 7) is emitted first and opens its bank with one full-bank start,
  everything else accumulates. Output drains in 3 pieces as column
  regions complete; the host does the final divide+transpose+scatter.
"""

import sys

if "/opt/trn_rl_repo" not in sys.path:
    sys.path.insert(0, "/opt/trn_rl_repo")

import numpy as np

B, S, D, H = 4, 2048, 1024, 64
P = 128
KO = D // P          # 8 dmodel chunks
NT = S // P          # 16 seq tiles
NEG = -30000.0
OWN0 = [0, 2, 4, 6, 9, 11, 13, 15]   # h=0 query tiles
OWN1 = [t for t in range(16) if t not in OWN0]


def _order(h):
    own = OWN0 if h == 0 else OWN1
    other = OWN1 if h == 0 else OWN0
    return sorted(own, reverse=True) + sorted(other)


def _width(p):
    return p + 1 if p < 8 else 16 - p


def _build_program(zb):
    import concourse.bacc as bacc
    import concourse.mybir as mybir
    import concourse.tile as tile

    f32 = mybir.dt.float32
    bf16 = mybir.dt.bfloat16
    AF = mybir.ActivationFunctionType
    ALU = mybir.AluOpType

    nc = bacc.Bacc()
    et = nc.declare_dram_parameter("et", [4, P, KO * 512], bf16, isOutput=False)
    # per partition cols 0:1536 = [Wv|Wk|Wq/8] x 8 ko, cols 1536:5632 = chunk0
    wc0 = nc.declare_dram_parameter("wc0", [P, 1536 + 4096], bf16, isOutput=False)
    # cols: bq/8 | bk | bv | bg[8..15] (0 or NEG per core)
    biasg = nc.declare_dram_parameter("biasg", [P, 11], f32, isOutput=False)
    # cols 0:128 = shared tri diag mask, cols 128:192 = identity (rows 0:64)
    mi = nc.declare_dram_parameter("mi", [P, P + H], bf16, isOutput=False)
    out = nc.declare_dram_parameter("out", [H + 1, 1024], f32, isOutput=True)

    from contextlib import ExitStack

    with tile.TileContext(nc) as tc, ExitStack() as ctx:
        cpool = ctx.enter_context(tc.tile_pool(name="const", bufs=1))
        vtp = ctx.enter_context(tc.tile_pool(name="vt", bufs=2))
        ptp = ctx.enter_context(tc.tile_pool(name="pt", bufs=10))
        psb = ctx.enter_context(tc.tile_pool(name="psb", bufs=2, space="PSUM"))

        # --- input DMAs: ONE hardware ring, strict need-order (both DGE
        # rings share HBM bandwidth; splitting starves the critical pieces)
        wc_sb = cpool.tile([P, 1536 + 4096], bf16, tag="wc0")
        nc.sync.dma_start(wc_sb[:, 0:1536], wc0[:, 0:1536])
        nc.sync.dma_start(wc_sb[:, 1536:4096], wc0[:, 1536:4096])
        nc.sync.dma_start(wc_sb[:, 4096:5632], wc0[:, 4096:5632])
        mi_sb = cpool.tile([P, P + H], bf16, tag="mi")
        nc.sync.dma_start(mi_sb[:], mi[:])
        bias_sb = cpool.tile([P, 11], f32, tag="biasg")
        nc.sync.dma_start(bias_sb[:], biasg[:])
        ET = cpool.tile([P, 4, KO, 512], bf16, tag="ET")
        nc.sync.dma_start(ET[:, 1, :, :], et[1, :, :])
        nc.sync.dma_start(ET[:, 2, :, :], et[2, :, :])
        nc.sync.dma_start(ET[:, 3, :, :], et[3, :, :])

        def w_ap(ko, a, b):
            return wc_sb[:, ko * 192 + a:ko * 192 + b]

        def et_ap(cc, ko):
            if cc == 0:
                return wc_sb[:, 1536 + ko * 512:1536 + (ko + 1) * 512]
            return ET[:, cc, ko, :]

        bq_sb = bias_sb[:, 0:1]
        bk_sb = bias_sb[:, 1:2]
        bv_sb = bias_sb[:H, 2:3]

        def bg_sb(p):
            return bias_sb[:, 3 + (p - 8):4 + (p - 8)]

        tri_sb = mi_sb[:, 0:P]
        id_sb = mi_sb[:H, P:P + H]

        QT = cpool.tile([P, 1024], bf16, tag="QT")
        KT = cpool.tile([P, S], bf16, tag="KT")
        Vp = cpool.tile([P, NT, H + 1], bf16, tag="Vp")
        o_sb = cpool.tile([H + 1, 1024], f32, tag="osb")
        wtile = cpool.tile([P, 512], bf16, tag="warm")
        nc.vector.memset(wtile[:], 0.0)
        nc.vector.memset(Vp[:, :, H:H + 1], 1.0)

        def vtranspose(vt, cc):
            for t in range(4):
                kt = cc * 4 + t
                pvt = psb.tile([P, H], bf16, tag="pj", name=f"pvt_{kt}")
                nc.tensor.transpose(
                    pvt[:], vt[:, t * P:(t + 1) * P], id_sb[:]
                )
                nc.vector.tensor_copy(Vp[:, kt, :H], pvt[:])

        vts = [None] * 4

        def pcopy(dst, src_ap, bias, eng):
            if zb:
                if eng == "act":
                    nc.scalar.activation(dst, src_ap, AF.Copy)
                else:
                    nc.vector.tensor_copy(dst, src_ap)
            else:
                nc.vector.tensor_scalar_add(dst, src_ap, bias)

        def vk_chunk(cc):
            ps = psb.tile([P, 512], f32, tag="pj", name=f"vk_ps_{cc}")
            for ko in range(KO):
                nc.tensor.matmul(
                    ps[:], w_ap(ko, 0, 128), et_ap(cc, ko),
                    start=(ko == 0), stop=(ko == KO - 1),
                )
            eng = "act" if cc < 1 else "dve"
            pcopy(
                KT[H:P, cc * 512:(cc + 1) * 512], ps[H:P, :], bk_sb[H:P], eng
            )
            vt = vtp.tile([H, 512], bf16, tag="vt", name=f"vt_{cc}")
            pcopy(vt[:], ps[:H, :], bv_sb[:], eng)
            vts[cc] = vt

        def q_chunk(cc):
            ps = psb.tile([P, 512], f32, tag="pj", name=f"q_ps_{cc}")
            for ko in range(KO):
                nc.tensor.matmul(
                    ps[H:P, :], w_ap(ko, 128, 192), et_ap(cc, ko),
                    start=(ko == 0), stop=(ko == KO - 1),
                )
            pcopy(
                QT[H:P, cc * 512:(cc + 1) * 512], ps[H:P, :], bq_sb[H:P],
                "act" if cc == 0 else "dve",
            )

        # --- attention: 16 prefix-range units over one 2-bank out^T psum
        outT = psb.tile([P, 1024], f32, tag="os", bufs=1)
        pts = [None] * NT

        def col_pieces(w128, bound=512):
            # split [0, w128) at the 512-col psum bank boundary
            if w128 <= bound:
                return [(0, w128)]
            return [(0, bound), (bound, w128)]

        def scores(p):
            w = _width(p) * P
            ps = psb.tile([P, 1024], f32, tag="sc", name=f"sc_{p}", bufs=2)
            pt = ptp.tile([P, 1024], bf16, tag="pt", name=f"pt_{p}")
            pts[p] = pt
            kblk = KT[H:P, p * P:(p + 1) * P]
            for a, b in col_pieces(w):
                nc.tensor.matmul(
                    ps[:, a:b], kblk, QT[H:P, a:b],
                    start=True, stop=True, skip_group_check=True,
                )
            if p < 8:
                # own key: exp all, tri-mask the diagonal (last) block
                for a, b in col_pieces(w):
                    nc.scalar.activation(pt[:, a:b], ps[:, a:b], AF.Exp)
                nc.vector.tensor_tensor(
                    pt[:, w - P:w], pt[:, w - P:w], tri_sb, ALU.mult
                )
            else:
                # other key: last block fully causal or fully dead
                # (0/-30000 per-core exp bias)
                if w > P:
                    for a, b in col_pieces(w - P):
                        nc.scalar.activation(pt[:, a:b], ps[:, a:b], AF.Exp)
                nc.scalar.activation(
                    pt[:, w - P:w], ps[:, w - P:w], AF.Exp, bias=bg_sb(p)
                )

        def pv(p, stops=()):
            # start=True resets the ENTIRE 512-col psum bank, so each bank
            # gets exactly one start: unit 3 opens bank A with its full
            # [0:512] write, unit 7 opens bank B with [512:1024]; they are
            # emitted before any other writer of their bank.
            w = _width(p) * P
            pt = pts[p]
            if p == 3:
                pieces = [(0, 512, True)]
            elif p == 7:
                pieces = [(0, 512, False), (512, 1024, True)]
            else:
                pieces = [(a, b, False) for a, b in col_pieces(w)]
            for a, b, st in pieces:
                nc.tensor.matmul(
                    outT[:H + 1, a:b], Vp[:, p, :], pt[:, a:b],
                    start=st, stop=(a in stops),
                    skip_group_check=True,
                )

        def drain(a, b):
            nc.vector.tensor_copy(o_sb[:, a:b], outT[:H + 1, a:b])
            nc.sync.dma_start(out[:, a:b], o_sb[:, a:b])

        # --- emission order = per-engine FIFO order ---
        # 13 back-to-back N=512 warmups run dense from ~8.3us THROUGH the
        # weights-DMA landing (~12.7us) so the HAM utilization window never
        # dips: the full-clock grant opens just before projections start
        # and, with sustained utilization, stays open through attention.
        for i in range(13):
            wps = psb.tile([P, 512], f32, tag="pj", name=f"warm_{i}")
            nc.tensor.matmul(
                wps[:], wtile[:, 0:P], wtile[:],
                start=True, stop=True, skip_group_check=True,
            )

        # transposes and ready pvs are placed to fill the PE bubble while
        # each chunk's PSUM->SBUF copies (ACT/DVE) land
        vk_chunk(0)
        q_chunk(0)
        vtranspose(vts[0], 0)
        scores(3)
        scores(0)
        pv(3)
        scores(1)
        pv(0)
        scores(2)
        pv(1)
        pv(2)
        vk_chunk(1)
        q_chunk(1)
        vtranspose(vts[1], 1)
        scores(7)
        scores(4)
        pv(7)
        scores(5)
        pv(4)
        scores(6)
        pv(5)
        pv(6)
        vk_chunk(2)
        vtranspose(vts[2], 2)
        scores(8)
        scores(9)
        pv(8)
        scores(10)
        pv(9)
        drain(768, 1024)
        scores(11)
        pv(10)
        vk_chunk(3)
        pv(11, stops=(512,))
        vtranspose(vts[3], 3)
        scores(12)
        scores(13)
        pv(12)
        scores(14)
        pv(13)
        drain(256, 768)
        scores(15)
        pv(14)
        pv(15, stops=(0,))
        drain(0, 256)

    nc.finalize()
    return nc


_CACHED = None


def _get_program(zb):
    global _CACHED
    if _CACHED is None or _CACHED[0] != zb:
        _CACHED = (zb, _build_program(zb))
    return _CACHED[1]


def _host_inputs(embeddings, Wq, bq, Wk, bk, Wv, bv):
    import ml_dtypes

    bf16 = ml_dtypes.bfloat16
    tri = np.zeros((P, P), np.float32)
    for k in range(P):
        tri[k, k:] = 1.0
    ident = np.zeros((P, H), np.float32)
    ident[:H] = np.eye(H, dtype=np.float32)
    mi = np.ascontiguousarray(
        np.concatenate([tri, ident], axis=1)
    ).astype(bf16)

    def wlay(w):
        return np.asarray(w, np.float32).reshape(KO, P, H).transpose(1, 0, 2)

    wq8l = wlay(Wq) / 8.0
    wkl = wlay(Wk)
    wvl = wlay(Wv)
    wts = np.concatenate([wvl, wkl, wq8l], axis=2).reshape(P, 1536)
    bqf = np.asarray(bq, np.float32) / 8.0
    bkf = np.asarray(bk, np.float32)
    bvf = np.asarray(bv, np.float32)
    z64 = np.zeros(H, np.float32)
    bq8P = np.concatenate([z64, bqf])
    bkP = np.concatenate([z64, bkf])
    bvP = np.concatenate([bvf, z64])

    in_maps = []
    perms = []
    for c in range(8):
        b, h = c // 2, c % 2
        order = _order(h)
        own = set(OWN0 if h == 0 else OWN1)
        rows = np.concatenate(
            [np.arange(t * P, (t + 1) * P) for t in order]
        )
        perms.append(rows)
        ep = embeddings[b][rows]                      # [S, D] f32, permuted
        etl = np.ascontiguousarray(
            ep.T.reshape(KO, P, 4, 512).transpose(2, 1, 0, 3)
        ).astype(bf16).reshape(4, P, KO * 512)        # [cc, p, ko*512]
        # bg[p]: 0 if the last block of unit p is fully causal, NEG if dead
        bgs = []
        for p in range(8, 16):
            key = order[p]
            s = sum(1 for t in own if t >= key)
            bgs.append(
                np.full(P, 0.0 if s == _width(p) else NEG, np.float32)
            )
        biasg = np.ascontiguousarray(
            np.stack([bq8P, bkP, bvP] + bgs, axis=1)
        )
        wc0l = np.ascontiguousarray(
            np.concatenate([wts, etl[0]], axis=1)
        ).astype(bf16)
        in_maps.append({
            "et": etl, "wc0": wc0l, "biasg": biasg, "mi": mi,
        })
    return in_maps, perms


def _run(embeddings, Wq, bq, Wk, bk, Wv, bv, trace=False):
    from concourse.bass_utils import run_bass_kernel_spmd

    zb = (
        not np.any(np.asarray(bq)) and not np.any(np.asarray(bk))
        and not np.any(np.asarray(bv))
    )
    nc = _get_program(zb)
    in_maps, perms = _host_inputs(embeddings, Wq, bq, Wk, bk, Wv, bv)
    res = run_bass_kernel_spmd(
        nc, in_maps, core_ids=list(range(8)), trace=trace,
        trace_cores=list(range(8)) if trace else None,
    )
    full = np.empty((B, S, H), np.float32)
    for c in range(8):
        b = c // 2
        o = res.results[c]["out"]                     # [65, 1024] f32
        full[b, perms[c][:1024]] = (o[:H] / o[H:H + 1]).T
    return full, res


def kernel(embeddings, Wq, bq, Wk, bk, Wv, bv):
    full, _ = _run(
        np.asarray(embeddings, np.float32), Wq, bq, Wk, bk, Wv, bv, trace=False
    )
    return full


# revision 26
# speedup vs baseline: 1.1322x; 1.0062x over previous
"""Causal single-head attention on 8 trn2 NeuronCores - split-72 geometry.

B=4, S=2048, D_MODEL=1024, D_HEAD=64, fp32 in/out.

Sharding: 2 cores per batch with an interleaved query-tile split
(h=0 owns tiles {0,2,4,6,9,11,13,15}, h=1 the complement; 68 causal
128x128 blocks each). The host feeds each core E^T [dm, s] bf16 with
columns ordered [own tiles DESCENDING | other tiles ascending]. With
own-descending query columns, the queries needing key tile at position
p form a PREFIX of the 1024 QT columns, so each score unit computes a
prefix range:
  position p 0..7  (own keys):   width (p+1)*128, diag tri at last block
  position p 8..15 (other keys): width (16-p)*128, last block either
    fully causal or fully dead - killed by a per-core 0/-30000 exp bias
Total 72 blocks/core vs 84 for the contiguous-half split (68 = ideal).

Per-core pipeline (identical SPMD program, all matmuls bf16):
  Warmup N=512 matmuls open the PE HAM clock gate while the first input
  DMA is in flight; inputs stream over BOTH hardware DGE rings (Sync +
  Act). Projections per 512-col chunk of E^T: one [Wv|Wk]-packed pass
  (V^T on PSUM rows 0:64, K^T on 64:128) plus, for the core's own 2
  chunks, a Wq/8 pass targeting PSUM rows 64:128. Q^T/K^T live on SBUF
  partitions 64:128 (shared-base-partition rule); V tiles are
  PE-transposed into Vp [128k, 16, 65] with a ones column (softmax
  denominator). Zero biases -> all projection PSUM->SBUF copies are
  bias-free.
  PV accumulates out^T [65, 1024] in one 2-bank PSUM tile; start=True
  resets a whole 512-col psum bank, so the widest unit of each bank
  (3, 7) is emitted first and opens its bank with one full-bank start,
  everything else accumulates. Output drains in 3 pieces as column
  regions complete; the host does the final divide+transpose+scatter.
"""

import sys

if "/opt/trn_rl_repo" not in sys.path:
    sys.path.insert(0, "/opt/trn_rl_repo")

import numpy as np

B, S, D, H = 4, 2048, 1024, 64
P = 128
KO = D // P          # 8 dmodel chunks
NT = S // P          # 16 seq tiles
NEG = -30000.0
OWN0 = [0, 2, 4, 6, 9, 11, 13, 15]   # h=0 query tiles
OWN1 = [t for t in range(16) if t not in OWN0]


def _order(h):
    own = OWN0 if h == 0 else OWN1
    other = OWN1 if h == 0 else OWN0
    return sorted(own, reverse=True) + sorted(other)


def _width(p):
    return p + 1 if p < 8 else 16 - p


def _build_program(zb):
    import concourse.bacc as bacc
    import concourse.mybir as mybir
    import concourse.tile as tile

    f32 = mybir.dt.float32
    bf16 = mybir.dt.bfloat16
    AF = mybir.ActivationFunctionType
    ALU = mybir.AluOpType

    nc = bacc.Bacc()
    et = nc.declare_dram_parameter("et", [4, P, KO * 512], bf16, isOutput=False)
    # per partition cols 0:1536 = [Wv|Wk|Wq/8] x 8 ko, cols 1536:5632 = chunk0
    wc0 = nc.declare_dram_parameter("wc0", [P, 1536 + 4096], bf16, isOutput=False)
    # cols: bq/8 | bk | bv | bg[8..15] (0 or NEG per core)
    biasg = nc.declare_dram_parameter("biasg", [P, 11], f32, isOutput=False)
    # cols 0:128 = shared tri diag mask, cols 128:192 = identity (rows 0:64)
    mi = nc.declare_dram_parameter("mi", [P, P + H], bf16, isOutput=False)
    out = nc.declare_dram_parameter("out", [H + 1, 1024], f32, isOutput=True)

    from contextlib import ExitStack

    with tile.TileContext(nc) as tc, ExitStack() as ctx:
        cpool = ctx.enter_context(tc.tile_pool(name="const", bufs=1))
        vtp = ctx.enter_context(tc.tile_pool(name="vt", bufs=2))
        ptp = ctx.enter_context(tc.tile_pool(name="pt", bufs=10))
        psb = ctx.enter_context(tc.tile_pool(name="psb", bufs=2, space="PSUM"))

        # --- input DMAs: ONE hardware ring, strict need-order (both DGE
        # rings share HBM bandwidth; splitting starves the critical pieces)
        wc_sb = cpool.tile([P, 1536 + 4096], bf16, tag="wc0")
        nc.sync.dma_start(wc_sb[:, 0:1536], wc0[:, 0:1536])
        # chunk0 in four pieces so the first projection ko's unblock as
        # soon as their slice lands (dma deps are per-dma_start)
        for a in range(1536, 5632, 1024):
            b = min(a + 1024, 5632)
            nc.sync.dma_start(wc_sb[:, a:b], wc0[:, a:b])
        mi_sb = cpool.tile([P, P + H], bf16, tag="mi")
        nc.sync.dma_start(mi_sb[:], mi[:])
        bias_sb = cpool.tile([P, 11], f32, tag="biasg")
        nc.sync.dma_start(bias_sb[:], biasg[:])
        ET = cpool.tile([P, 4, KO, 512], bf16, tag="ET")
        nc.sync.dma_start(ET[:, 1, :, :], et[1, :, :])
        nc.sync.dma_start(ET[:, 2, :, :], et[2, :, :])
        nc.sync.dma_start(ET[:, 3, :, :], et[3, :, :])

        def w_ap(ko, a, b):
            return wc_sb[:, ko * 192 + a:ko * 192 + b]

        def et_ap(cc, ko):
            if cc == 0:
                return wc_sb[:, 1536 + ko * 512:1536 + (ko + 1) * 512]
            return ET[:, cc, ko, :]

        bq_sb = bias_sb[:, 0:1]
        bk_sb = bias_sb[:, 1:2]
        bv_sb = bias_sb[:H, 2:3]

        def bg_sb(p):
            return bias_sb[:, 3 + (p - 8):4 + (p - 8)]

        tri_sb = mi_sb[:, 0:P]
        id_sb = mi_sb[:H, P:P + H]

        QT = cpool.tile([P, 1024], bf16, tag="QT")
        KT = cpool.tile([P, S], bf16, tag="KT")
        Vp = cpool.tile([P, NT, H + 1], bf16, tag="Vp")
        o_sb = cpool.tile([H + 1, 1024], f32, tag="osb")
        wtile = cpool.tile([P, 512], bf16, tag="warm")
        nc.vector.memset(wtile[:], 0.0)
        nc.vector.memset(Vp[:, :, H:H + 1], 1.0)

        def vtranspose(vt, cc):
            for t in range(4):
                kt = cc * 4 + t
                pvt = psb.tile([P, H], bf16, tag="pj", name=f"pvt_{kt}")
                nc.tensor.transpose(
                    pvt[:], vt[:, t * P:(t + 1) * P], id_sb[:]
                )
                nc.vector.tensor_copy(Vp[:, kt, :H], pvt[:])

        vts = [None] * 4

        def pcopy(dst, src_ap, bias, eng):
            if zb:
                if eng == "act":
                    nc.scalar.activation(dst, src_ap, AF.Copy)
                else:
                    nc.vector.tensor_copy(dst, src_ap)
            else:
                nc.vector.tensor_scalar_add(dst, src_ap, bias)

        def vk_chunk(cc, halves=False):
            # halves=True: two independent 256-col accumulation groups (in
            # SEPARATE psum tiles - start=True resets a whole bank) so the
            # first half's PSUM->SBUF copies overlap the second half's
            # matmuls - removes the chunk-transition PE bubble
            eng = "act" if cc < 1 else "dve"
            vt = vtp.tile([H, 512], bf16, tag="vt", name=f"vt_{cc}")
            grps = [(0, 256), (256, 512)] if halves else [(0, 512)]
            for a, b in grps:
                ps = psb.tile([P, b - a], f32, tag="pj",
                              name=f"vk_ps_{cc}_{a}")
                for ko in range(KO):
                    nc.tensor.matmul(
                        ps[:], w_ap(ko, 0, 128), et_ap(cc, ko)[:, a:b],
                        start=(ko == 0), stop=(ko == KO - 1),
                        skip_group_check=True,
                    )
                pcopy(
                    KT[H:P, cc * 512 + a:cc * 512 + b], ps[H:P, :],
                    bk_sb[H:P], eng,
                )
                pcopy(vt[:, a:b], ps[:H, :], bv_sb[:], eng)
            vts[cc] = vt

        def q_chunk(cc):
            ps = psb.tile([P, 512], f32, tag="pj", name=f"q_ps_{cc}")
            for ko in range(KO):
                nc.tensor.matmul(
                    ps[H:P, :], w_ap(ko, 128, 192), et_ap(cc, ko),
                    start=(ko == 0), stop=(ko == KO - 1),
                )
            pcopy(
                QT[H:P, cc * 512:(cc + 1) * 512], ps[H:P, :], bq_sb[H:P],
                "act" if cc == 0 else "dve",
            )

        # --- attention: 16 prefix-range units over one 2-bank out^T psum
        outT = psb.tile([P, 1024], f32, tag="os", bufs=1)
        pts = [None] * NT

        def col_pieces(w128, bound=512):
            # split [0, w128) at the 512-col psum bank boundary
            if w128 <= bound:
                return [(0, w128)]
            return [(0, bound), (bound, w128)]

        def scores(p):
            w = _width(p) * P
            ps = psb.tile([P, 1024], f32, tag="sc", name=f"sc_{p}", bufs=2)
            pt = ptp.tile([P, 1024], bf16, tag="pt", name=f"pt_{p}")
            pts[p] = pt
            kblk = KT[H:P, p * P:(p + 1) * P]
            for a, b in col_pieces(w):
                nc.tensor.matmul(
                    ps[:, a:b], kblk, QT[H:P, a:b],
                    start=True, stop=True, skip_group_check=True,
                )
            if p < 8:
                # own key: exp all, tri-mask the diagonal (last) block
                for a, b in col_pieces(w):
                    nc.scalar.activation(pt[:, a:b], ps[:, a:b], AF.Exp)
                nc.vector.tensor_tensor(
                    pt[:, w - P:w], pt[:, w - P:w], tri_sb, ALU.mult
                )
            else:
                # other key: last block fully causal or fully dead
                # (0/-30000 per-core exp bias)
                if w > P:
                    for a, b in col_pieces(w - P):
                        nc.scalar.activation(pt[:, a:b], ps[:, a:b], AF.Exp)
                nc.scalar.activation(
                    pt[:, w - P:w], ps[:, w - P:w], AF.Exp, bias=bg_sb(p)
                )

        def pv(p, stops=()):
            # start=True resets the ENTIRE 512-col psum bank, so each bank
            # gets exactly one start: unit 3 opens bank A with its full
            # [0:512] write, unit 7 opens bank B with [512:1024]; they are
            # emitted before any other writer of their bank.
            w = _width(p) * P
            pt = pts[p]
            if p == 3:
                pieces = [(0, 512, True)]
            elif p == 7:
                pieces = [(0, 512, False), (512, 1024, True)]
            else:
                pieces = [(a, b, False) for a, b in col_pieces(w)]
            for a, b, st in pieces:
                nc.tensor.matmul(
                    outT[:H + 1, a:b], Vp[:, p, :], pt[:, a:b],
                    start=st, stop=(a in stops),
                    skip_group_check=True,
                )

        def drain(a, b):
            nc.vector.tensor_copy(o_sb[:, a:b], outT[:H + 1, a:b])
            nc.sync.dma_start(out[:, a:b], o_sb[:, a:b])

        # --- emission order = per-engine FIFO order ---
        # 13 back-to-back N=512 warmups run dense from ~8.3us THROUGH the
        # weights-DMA landing (~12.7us) so the HAM utilization window never
        # dips: the full-clock grant opens just before projections start
        # and, with sustained utilization, stays open through attention.
        for i in range(15):
            wps = psb.tile([P, 512], f32, tag="pj", name=f"warm_{i}")
            nc.tensor.matmul(
                wps[:], wtile[:, 0:P], wtile[:],
                start=True, stop=True, skip_group_check=True,
            )

        # transposes and ready pvs are placed to fill the PE bubble while
        # each chunk's PSUM->SBUF copies (ACT/DVE) land
        vk_chunk(0)
        q_chunk(0)
        vtranspose(vts[0], 0)
        scores(3)
        scores(0)
        pv(3)
        scores(1)
        pv(0)
        scores(2)
        pv(1)
        pv(2)
        vk_chunk(1)
        q_chunk(1)
        vtranspose(vts[1], 1)
        scores(7)
        scores(4)
        pv(7)
        scores(5)
        pv(4)
        scores(6)
        pv(5)
        pv(6)
        vk_chunk(2, halves=True)
        vtranspose(vts[2], 2)
        scores(8)
        scores(9)
        pv(8)
        scores(10)
        pv(9)
        drain(768, 1024)
        scores(11)
        pv(10)
        vk_chunk(3, halves=True)
        pv(11, stops=(512,))
        vtranspose(vts[3], 3)
        scores(12)
        scores(13)
        pv(12)
        scores(14)
        pv(13)
        drain(256, 768)
        scores(15)
        pv(14)
        drain(128, 256)
        pv(15, stops=(0,))
        drain(0, 128)

    nc.finalize()
    return nc


_CACHED = None


def _get_program(zb):
    global _CACHED
    if _CACHED is None or _CACHED[0] != zb:
        _CACHED = (zb, _build_program(zb))
    return _CACHED[1]


def _host_inputs(embeddings, Wq, bq, Wk, bk, Wv, bv):
    import ml_dtypes

    bf16 = ml_dtypes.bfloat16
    tri = np.zeros((P, P), np.float32)
    for k in range(P):
        tri[k, k:] = 1.0
    ident = np.zeros((P, H), np.float32)
    ident[:H] = np.eye(H, dtype=np.float32)
    mi = np.ascontiguousarray(
        np.concatenate([tri, ident], axis=1)
    ).astype(bf16)

    def wlay(w):
        return np.asarray(w, np.float32).reshape(KO, P, H).transpose(1, 0, 2)

    wq8l = wlay(Wq) / 8.0
    wkl = wlay(Wk)
    wvl = wlay(Wv)
    wts = np.concatenate([wvl, wkl, wq8l], axis=2).reshape(P, 1536)
    bqf = np.asarray(bq, np.float32) / 8.0
    bkf = np.asarray(bk, np.float32)
    bvf = np.asarray(bv, np.float32)
    z64 = np.zeros(H, np.float32)
    bq8P = np.concatenate([z64, bqf])
    bkP = np.concatenate([z64, bkf])
    bvP = np.concatenate([bvf, z64])

    in_maps = []
    perms = []
    for c in range(8):
        b, h = c // 2, c % 2
        order = _order(h)
        own = set(OWN0 if h == 0 else OWN1)
        rows = np.concatenate(
            [np.arange(t * P, (t + 1) * P) for t in order]
        )
        perms.append(rows)
        ep = embeddings[b][rows]                      # [S, D] f32, permuted
        etl = np.ascontiguousarray(
            ep.T.reshape(KO, P, 4, 512).transpose(2, 1, 0, 3)
        ).astype(bf16).reshape(4, P, KO * 512)        # [cc, p, ko*512]
        # bg[p]: 0 if the last block of unit p is fully causal, NEG if dead
        bgs = []
        for p in range(8, 16):
            key = order[p]
            s = sum(1 for t in own if t >= key)
            bgs.append(
                np.full(P, 0.0 if s == _width(p) else NEG, np.float32)
            )
        biasg = np.ascontiguousarray(
            np.stack([bq8P, bkP, bvP] + bgs, axis=1)
        )
        wc0l = np.ascontiguousarray(
            np.concatenate([wts, etl[0]], axis=1)
        ).astype(bf16)
        in_maps.append({
            "et": etl, "wc0": wc0l, "biasg": biasg, "mi": mi,
        })
    return in_maps, perms


def _run(embeddings, Wq, bq, Wk, bk, Wv, bv, trace=False):
    from concourse.bass_utils import run_bass_kernel_spmd

    zb = (
        not np.any(np.asarray(bq)) and not np.any(np.asarray(bk))
        and not np.any(np.asarray(bv))
    )
    nc = _get_program(zb)
    in_maps, perms = _host_inputs(embeddings, Wq, bq, Wk, bk, Wv, bv)
    res = run_bass_kernel_spmd(
        nc, in_maps, core_ids=list(range(8)), trace=trace,
        trace_cores=list(range(8)) if trace else None,
    )
    full = np.empty((B, S, H), np.float32)
    for c in range(8):
        b = c // 2
        o = res.results[c]["out"]                     # [65, 1024] f32
        full[b, perms[c][:1024]] = (o[:H] / o[H:H + 1]).T
    return full, res


def kernel(embeddings, Wq, bq, Wk, bk, Wv, bv):
    full, _ = _run(
        np.asarray(embeddings, np.float32), Wq, bq, Wk, bk, Wv, bv, trace=False
    )
    return full


# revision 30
# speedup vs baseline: 1.1751x; 1.0378x over previous
"""Causal single-head attention on 8 trn2 NeuronCores - split-72 geometry.

B=4, S=2048, D_MODEL=1024, D_HEAD=64, fp32 in/out.

Sharding: 2 cores per batch with an interleaved query-tile split
(h=0 owns tiles {0,2,4,6,9,11,13,15}, h=1 the complement; 68 causal
128x128 blocks each). The host feeds each core E^T [dm, s] bf16 with
columns ordered [own tiles DESCENDING | other tiles ascending]. With
own-descending query columns, the queries needing key tile at position
p form a PREFIX of the 1024 QT columns, so each score unit computes a
prefix range:
  position p 0..7  (own keys):   width (p+1)*128, diag tri at last block
  position p 8..15 (other keys): width (16-p)*128, last block either
    fully causal or fully dead - killed by a per-core 0/-30000 exp bias
Total 72 blocks/core vs 84 for the contiguous-half split (68 = ideal).

Per-core pipeline (identical SPMD program, all matmuls bf16):
  Warmup N=512 matmuls open the PE HAM clock gate while the first input
  DMA is in flight; inputs stream over BOTH hardware DGE rings (Sync +
  Act). Projections per 512-col chunk of E^T: one [Wv|Wk]-packed pass
  (V^T on PSUM rows 0:64, K^T on 64:128) plus, for the core's own 2
  chunks, a Wq/8 pass targeting PSUM rows 64:128. Q^T/K^T live on SBUF
  partitions 64:128 (shared-base-partition rule); V tiles are
  PE-transposed into Vp [128k, 16, 65] with a ones column (softmax
  denominator). Zero biases -> all projection PSUM->SBUF copies are
  bias-free.
  PV accumulates out^T [65, 1024] in one 2-bank PSUM tile; start=True
  resets a whole 512-col psum bank, so the widest unit of each bank
  (3, 7) is emitted first and opens its bank with one full-bank start,
  everything else accumulates. Output drains in 3 pieces as column
  regions complete; the host does the final divide+transpose+scatter.
"""

import sys

if "/opt/trn_rl_repo" not in sys.path:
    sys.path.insert(0, "/opt/trn_rl_repo")

import numpy as np

B, S, D, H = 4, 2048, 1024, 64
P = 128
KO = D // P          # 8 dmodel chunks
NT = S // P          # 16 seq tiles
NEG = -30000.0
OWN0 = [0, 2, 4, 6, 9, 11, 13, 15]   # h=0 query tiles
OWN1 = [t for t in range(16) if t not in OWN0]


def _order(h):
    own = OWN0 if h == 0 else OWN1
    other = OWN1 if h == 0 else OWN0
    return sorted(own, reverse=True) + sorted(other)


def _width(p):
    return p + 1 if p < 8 else 16 - p


def _build_program(zb):
    import concourse.bacc as bacc
    import concourse.mybir as mybir
    import concourse.tile as tile

    f32 = mybir.dt.float32
    bf16 = mybir.dt.bfloat16
    AF = mybir.ActivationFunctionType
    ALU = mybir.AluOpType

    nc = bacc.Bacc()
    et = nc.declare_dram_parameter("et", [4, P, KO * 512], bf16, isOutput=False)
    # per partition cols 0:1536 = [Wv|Wk|Wq/8] x 8 ko, cols 1536:5632 = chunk0
    wc0 = nc.declare_dram_parameter("wc0", [P, 1536 + 4096], bf16, isOutput=False)
    # cols: bq/8 | bk | bv | bg[8..15] (0 or NEG per core)
    biasg = nc.declare_dram_parameter("biasg", [P, 11], f32, isOutput=False)
    # cols 0:128 = shared tri diag mask, cols 128:192 = identity (rows 0:64)
    mi = nc.declare_dram_parameter("mi", [P, P + H], bf16, isOutput=False)
    out = nc.declare_dram_parameter("out", [H + 1, 1024], f32, isOutput=True)

    from contextlib import ExitStack

    with tile.TileContext(nc) as tc, ExitStack() as ctx:
        cpool = ctx.enter_context(tc.tile_pool(name="const", bufs=1))
        vtp = ctx.enter_context(tc.tile_pool(name="vt", bufs=2))
        ptp = ctx.enter_context(tc.tile_pool(name="pt", bufs=10))
        psb = ctx.enter_context(tc.tile_pool(name="psb", bufs=2, space="PSUM"))

        # --- input DMAs: ONE hardware ring, strict need-order (both DGE
        # rings share HBM bandwidth; splitting starves the critical pieces)
        wc_sb = cpool.tile([P, 1536 + 4096], bf16, tag="wc0")
        nc.sync.dma_start(wc_sb[:, 0:1536], wc0[:, 0:1536])
        # chunk0 in four pieces so the first projection ko's unblock as
        # soon as their slice lands (dma deps are per-dma_start)
        for a in range(1536, 5632, 1024):
            b = min(a + 1024, 5632)
            nc.sync.dma_start(wc_sb[:, a:b], wc0[:, a:b])
        mi_sb = cpool.tile([P, P + H], bf16, tag="mi")
        nc.sync.dma_start(mi_sb[:], mi[:])
        bias_sb = cpool.tile([P, 11], f32, tag="biasg")
        nc.sync.dma_start(bias_sb[:], biasg[:])
        ET = cpool.tile([P, 4, KO, 512], bf16, tag="ET")
        nc.sync.dma_start(ET[:, 1, :, :], et[1, :, :])
        nc.sync.dma_start(ET[:, 2, :, :], et[2, :, :])
        nc.sync.dma_start(ET[:, 3, :, :], et[3, :, :])

        def w_ap(ko, a, b):
            return wc_sb[:, ko * 192 + a:ko * 192 + b]

        def et_ap(cc, ko):
            if cc == 0:
                return wc_sb[:, 1536 + ko * 512:1536 + (ko + 1) * 512]
            return ET[:, cc, ko, :]

        bq_sb = bias_sb[:, 0:1]
        bk_sb = bias_sb[:, 1:2]
        bv_sb = bias_sb[:H, 2:3]

        def bg_sb(p):
            return bias_sb[:, 3 + (p - 8):4 + (p - 8)]

        tri_sb = mi_sb[:, 0:P]
        id_sb = mi_sb[:H, P:P + H]

        QT = cpool.tile([P, 1024], bf16, tag="QT")
        KT = cpool.tile([P, S], bf16, tag="KT")
        Vp = cpool.tile([P, NT, H + 1], bf16, tag="Vp")
        o_sb = cpool.tile([H + 1, 1024], f32, tag="osb")
        wtile = cpool.tile([P, 512], bf16, tag="warm")
        nc.vector.memset(wtile[:], 0.0)
        nc.vector.memset(Vp[:, :, H:H + 1], 1.0)

        def vtranspose(vt, cc):
            for t in range(4):
                kt = cc * 4 + t
                pvt = psb.tile([P, H], bf16, tag="pj", name=f"pvt_{kt}")
                nc.tensor.transpose(
                    pvt[:], vt[:, t * P:(t + 1) * P], id_sb[:]
                )
                nc.vector.tensor_copy(Vp[:, kt, :H], pvt[:])

        vts = [None] * 4

        def pcopy(dst, src_ap, bias, eng):
            if zb:
                if eng == "act":
                    nc.scalar.activation(dst, src_ap, AF.Copy)
                else:
                    nc.vector.tensor_copy(dst, src_ap)
            else:
                nc.vector.tensor_scalar_add(dst, src_ap, bias)

        def vk_chunk(cc, halves=False):
            # halves=True: two independent 256-col accumulation groups (in
            # SEPARATE psum tiles - start=True resets a whole bank) so the
            # first half's PSUM->SBUF copies overlap the second half's
            # matmuls - removes the chunk-transition PE bubble
            # KT/vt copies on DVE so they run concurrently with q_chunk's
            # ACT copy - the first scores then wait max(DVE, ACT) not sum
            eng = "dve"
            vt = vtp.tile([H, 512], bf16, tag="vt", name=f"vt_{cc}")
            grps = [(0, 256), (256, 512)] if halves else [(0, 512)]
            for a, b in grps:
                ps = psb.tile([P, b - a], f32, tag="pj",
                              name=f"vk_ps_{cc}_{a}")
                for ko in range(KO):
                    nc.tensor.matmul(
                        ps[:], w_ap(ko, 0, 128), et_ap(cc, ko)[:, a:b],
                        start=(ko == 0), stop=(ko == KO - 1),
                        skip_group_check=True,
                    )
                pcopy(
                    KT[H:P, cc * 512 + a:cc * 512 + b], ps[H:P, :],
                    bk_sb[H:P], eng,
                )
                pcopy(vt[:, a:b], ps[:H, :], bv_sb[:], eng)
            vts[cc] = vt

        def q_chunk(cc):
            ps = psb.tile([P, 512], f32, tag="pj", name=f"q_ps_{cc}")
            for ko in range(KO):
                nc.tensor.matmul(
                    ps[H:P, :], w_ap(ko, 128, 192), et_ap(cc, ko),
                    start=(ko == 0), stop=(ko == KO - 1),
                )
            pcopy(
                QT[H:P, cc * 512:(cc + 1) * 512], ps[H:P, :], bq_sb[H:P],
                "act" if cc == 0 else "dve",
            )

        # --- attention: 16 prefix-range units over one 2-bank out^T psum
        outT = psb.tile([P, 1024], f32, tag="os", bufs=1)
        pts = [None] * NT

        def col_pieces(w128, bound=512):
            # split [0, w128) at the 512-col psum bank boundary
            if w128 <= bound:
                return [(0, w128)]
            return [(0, bound), (bound, w128)]

        def scores(p):
            w = _width(p) * P
            ps = psb.tile([P, 1024], f32, tag="sc", name=f"sc_{p}", bufs=2)
            pt = ptp.tile([P, 1024], bf16, tag="pt", name=f"pt_{p}")
            pts[p] = pt
            kblk = KT[H:P, p * P:(p + 1) * P]
            for a, b in col_pieces(w):
                nc.tensor.matmul(
                    ps[:, a:b], kblk, QT[H:P, a:b],
                    start=True, stop=True, skip_group_check=True,
                )
            if p < 8:
                # own key: exp all, tri-mask the diagonal (last) block
                for a, b in col_pieces(w):
                    nc.scalar.activation(pt[:, a:b], ps[:, a:b], AF.Exp)
                nc.vector.tensor_tensor(
                    pt[:, w - P:w], pt[:, w - P:w], tri_sb, ALU.mult
                )
            else:
                # other key: last block fully causal or fully dead
                # (0/-30000 per-core exp bias)
                if w > P:
                    for a, b in col_pieces(w - P):
                        nc.scalar.activation(pt[:, a:b], ps[:, a:b], AF.Exp)
                nc.scalar.activation(
                    pt[:, w - P:w], ps[:, w - P:w], AF.Exp, bias=bg_sb(p)
                )

        def pv(p, stops=()):
            # start=True resets the ENTIRE 512-col psum bank, so each bank
            # gets exactly one start: unit 3 opens bank A with its full
            # [0:512] write, unit 7 opens bank B with [512:1024]; they are
            # emitted before any other writer of their bank.
            w = _width(p) * P
            pt = pts[p]
            if p == 3:
                pieces = [(0, 512, True)]
            elif p == 7:
                pieces = [(0, 512, False), (512, 1024, True)]
            else:
                pieces = [(a, b, False) for a, b in col_pieces(w)]
            for a, b, st in pieces:
                nc.tensor.matmul(
                    outT[:H + 1, a:b], Vp[:, p, :], pt[:, a:b],
                    start=st, stop=(a in stops),
                    skip_group_check=True,
                )

        def drain(a, b):
            nc.vector.tensor_copy(o_sb[:, a:b], outT[:H + 1, a:b])
            nc.sync.dma_start(out[:, a:b], o_sb[:, a:b])

        # --- emission order = per-engine FIFO order ---
        # 13 back-to-back N=512 warmups run dense from ~8.3us THROUGH the
        # weights-DMA landing (~12.7us) so the HAM utilization window never
        # dips: the full-clock grant opens just before projections start
        # and, with sustained utilization, stays open through attention.
        for i in range(15):
            wps = psb.tile([P, 512], f32, tag="pj", name=f"warm_{i}")
            nc.tensor.matmul(
                wps[:], wtile[:, 0:P], wtile[:],
                start=True, stop=True, skip_group_check=True,
            )

        # transposes and ready pvs are placed to fill the PE bubble while
        # each chunk's PSUM->SBUF copies (ACT/DVE) land
        vk_chunk(0)
        q_chunk(0)
        vtranspose(vts[0], 0)
        scores(3)
        scores(0)
        pv(3)
        scores(1)
        pv(0)
        scores(2)
        pv(1)
        pv(2)
        vk_chunk(1)
        q_chunk(1)
        vtranspose(vts[1], 1)
        scores(7)
        scores(4)
        pv(7)
        scores(5)
        pv(4)
        scores(6)
        pv(5)
        pv(6)
        vk_chunk(2, halves=True)
        vtranspose(vts[2], 2)
        scores(8)
        scores(9)
        pv(8)
        scores(10)
        pv(9)
        drain(768, 1024)
        scores(11)
        pv(10)
        vk_chunk(3, halves=True)
        pv(11, stops=(512,))
        vtranspose(vts[3], 3)
        scores(12)
        scores(13)
        pv(12)
        scores(14)
        pv(13)
        drain(256, 768)
        scores(15)
        pv(14)
        pv(15, stops=(0,))
        drain(0, 256)

    nc.finalize()
    return nc


_CACHED = None


def _get_program(zb):
    global _CACHED
    if _CACHED is None or _CACHED[0] != zb:
        _CACHED = (zb, _build_program(zb))
    return _CACHED[1]


def _host_inputs(embeddings, Wq, bq, Wk, bk, Wv, bv):
    import ml_dtypes

    bf16 = ml_dtypes.bfloat16
    tri = np.zeros((P, P), np.float32)
    for k in range(P):
        tri[k, k:] = 1.0
    ident = np.zeros((P, H), np.float32)
    ident[:H] = np.eye(H, dtype=np.float32)
    mi = np.ascontiguousarray(
        np.concatenate([tri, ident], axis=1)
    ).astype(bf16)

    def wlay(w):
        return np.asarray(w, np.float32).reshape(KO, P, H).transpose(1, 0, 2)

    wq8l = wlay(Wq) / 8.0
    wkl = wlay(Wk)
    wvl = wlay(Wv)
    wts = np.concatenate([wvl, wkl, wq8l], axis=2).reshape(P, 1536)
    bqf = np.asarray(bq, np.float32) / 8.0
    bkf = np.asarray(bk, np.float32)
    bvf = np.asarray(bv, np.float32)
    z64 = np.zeros(H, np.float32)
    bq8P = np.concatenate([z64, bqf])
    bkP = np.concatenate([z64, bkf])
    bvP = np.concatenate([bvf, z64])

    in_maps = []
    perms = []
    for c in range(8):
        b, h = c // 2, c % 2
        order = _order(h)
        own = set(OWN0 if h == 0 else OWN1)
        rows = np.concatenate(
            [np.arange(t * P, (t + 1) * P) for t in order]
        )
        perms.append(rows)
        ep = embeddings[b][rows]                      # [S, D] f32, permuted
        etl = np.ascontiguousarray(
            ep.T.reshape(KO, P, 4, 512).transpose(2, 1, 0, 3)
        ).astype(bf16).reshape(4, P, KO * 512)        # [cc, p, ko*512]
        # bg[p]: 0 if the last block of unit p is fully causal, NEG if dead
        bgs = []
        for p in range(8, 16):
            key = order[p]
            s = sum(1 for t in own if t >= key)
            bgs.append(
                np.full(P, 0.0 if s == _width(p) else NEG, np.float32)
            )
        biasg = np.ascontiguousarray(
            np.stack([bq8P, bkP, bvP] + bgs, axis=1)
        )
        wc0l = np.ascontiguousarray(
            np.concatenate([wts, etl[0]], axis=1)
        ).astype(bf16)
        in_maps.append({
            "et": etl, "wc0": wc0l, "biasg": biasg, "mi": mi,
        })
    return in_maps, perms


def _run(embeddings, Wq, bq, Wk, bk, Wv, bv, trace=False):
    from concourse.bass_utils import run_bass_kernel_spmd

    zb = (
        not np.any(np.asarray(bq)) and not np.any(np.asarray(bk))
        and not np.any(np.asarray(bv))
    )
    nc = _get_program(zb)
    in_maps, perms = _host_inputs(embeddings, Wq, bq, Wk, bk, Wv, bv)
    res = run_bass_kernel_spmd(
        nc, in_maps, core_ids=list(range(8)), trace=trace,
        trace_cores=list(range(8)) if trace else None,
    )
    full = np.empty((B, S, H), np.float32)
    for c in range(8):
        b = c // 2
        o = res.results[c]["out"]                     # [65, 1024] f32
        full[b, perms[c][:1024]] = (o[:H] / o[H:H + 1]).T
    return full, res


def kernel(embeddings, Wq, bq, Wk, bk, Wv, bv):
    full, _ = _run(
        np.asarray(embeddings, np.float32), Wq, bq, Wk, bk, Wv, bv, trace=False
    )
    return full
